# revision 32
# baseline (speedup 1.0000x reference)
"""Trainium2 Bass kernel for nn_CLS_30562987278491 (Wiener-deconvolution net).

Self-contained: hardcodes shapes B=8, NF=64, C=16, H=W=246, ks=21, FFT N=288.
Sharding: data-parallel over batch B across the 8 NeuronCores (1 image/core).

Decomposition (validated stage-by-stage against the jax reference):
  - conv_red (1x1) as matmul over the channel dim (2048-pixel slabs).
  - 3x3 convs via the R=6 row-shift scheme.  conv1 gathers cls from DRAM;
    h1/h2 stay SBUF-RESIDENT in the gathered rhs layout [(c, rm 0..7), t, x]
    using 128-col zero-padded lhsT (PSUM partitions == rhs partitions, so the
    leaky write is partition-identity) + tiny partition-shift DMAs that fill
    rm 6,7 of slot t from rm 0,1 of slot t+1.  No h1/h2 DRAM round-trips.
  - adaptive pool 3x3 via a [240,3] ones-matmul + free-dim reduce.
  - FFT as DFT matmuls: edge-replication pad folded into Fpad [246,288];
    Hermitian half-spectrum (v < 145).  F2 runs in bf16 (1 cyc/row at 145
    free); the Wiener denominator |Pf|^2 comes from the 5x5 autocorrelation Q
    of kernel_P via f32r matmuls at 290 free (packed QE layout), and Kf2 is
    added on DVE (no identity-matmul).  Plane loop is split in two passes:
    pass A (F1/F2/numerator A = C*conj(Kf)) runs while the pool/kp/Q chain
    resolves; pass B (denominator, Z, IFFT, crop) follows.
  - conv_exp (1x1) as matmul over 2048-pixel slabs.

Perf notes (TimelineSim): 663us baseline -> 451us.  DMA_ENGINES is a single
exclusive resource (~360 GB/s, runs <512B pay 2x); SWDGE costs Pool 994ns
fixed per gpsimd DMA; HWDGE 625ns per sync DMA; f32 matmuls are 4 cyc/row,
f32r 1 cyc/row only at free>=256, bf16 1 cyc/row at any width.  bf16 DMA
(upload or SBUF-SBUF) is BROKEN in this container path - convert on device.
"""
import numpy as np
import ml_dtypes

import concourse.bass as bass
import concourse.bacc as bacc
import concourse.mybir as mybir
import concourse.tile as tile
from concourse.bass_utils import run_bass_kernel_spmd

F32 = mybir.dt.float32
DT = mybir.dt.float32r          # 4-byte, bit-compatible with f32; PE 1 cyc/row at free>=256
BF = mybir.dt.bfloat16
NP_DT = np.float32
NP_BF = ml_dtypes.bfloat16

B, NF, C, H = 8, 64, 16, 246
N = 288
VH = 145                    # N//2 + 1
KS = 21
NPIX = H * H                # 60516
CROP = 21


# ---------------------------------------------------------------- host consts
def _build_consts():
    cs = {}
    u = np.arange(N)
    v = np.arange(VH)
    F = np.exp(-2j * np.pi * np.outer(np.arange(N), u) / N)
    Fpad = np.zeros((H, N), complex)
    Fpad[0] = F[0:22].sum(0)
    Fpad[1:245] = F[22:266]
    Fpad[245] = F[266:288].sum(0)

    FuB = np.concatenate([Fpad.real, Fpad.imag], axis=1)        # [246, 576]
    cs['FuB'] = FuB.reshape(2, 123, 576).transpose(1, 0, 2)     # [123, 2, 576]

    def vchunk(m):                                              # [246,145] -> [123,2,145]
        return m.reshape(2, 123, VH).transpose(1, 0, 2)
    cs['Fvr'] = vchunk(Fpad[:, :VH].real)
    cs['Fvi'] = vchunk(Fpad[:, :VH].imag)
    cs['Fvn'] = vchunk(-Fpad[:, :VH].imag)

    d5 = np.arange(5) - 2
    E5v = np.exp(-2j * np.pi * np.outer(d5, v) / N)             # [5, 145]
    cs['E5v'] = np.concatenate([E5v.real, E5v.imag], axis=1)    # [5, 290]
    th5 = 2 * np.pi * np.outer(d5, u) / N                       # [5, 288]
    cs['E5uc'] = np.cos(th5).reshape(5, 3, 96)
    cs['E5us'] = np.sin(th5).reshape(5, 3, 96)

    d21 = np.arange(21) - 10
    E21u = np.exp(-2j * np.pi * np.outer(d21, u) / N)           # [21, 288]
    cs['E21u'] = np.concatenate([E21u.real, E21u.imag], axis=1)  # [21, 576]
    E21v = np.exp(-2j * np.pi * np.outer(d21, v) / N)           # [21, 145]
    z = np.zeros((21, 290))
    z[:, :VH] = E21v.real
    cs['E21vr'] = z.copy()
    z = np.zeros((21, 290))
    z[:, :VH] = E21v.imag
    cs['E21vi'] = z.copy()
    z = np.zeros((21, 290))
    z[:, :VH] = -E21v.imag
    cs['E21vin'] = z.copy()

    a = CROP + np.arange(256)
    thu = 2 * np.pi * np.outer(u, a) / N                        # [288, 256]
    cs['Eur'] = np.cos(thu).reshape(3, 96, 256).transpose(1, 0, 2)   # [96, 3, 256]
    cs['Eui'] = np.sin(thu).reshape(3, 96, 256).transpose(1, 0, 2)
    cs['Eurn'] = -cs['Eur']

    wv = np.where((v == 0) | (v == N // 2), 1.0, 2.0) / (N * N)
    bb = CROP + np.arange(256)
    thv = 2 * np.pi * np.outer(v, bb) / N                       # [145, 256]
    wEv_r = wv[:, None] * np.cos(thv)
    wEv_i = wv[:, None] * np.sin(thv)
    wEv_r[:, H:] = 0.0
    wEv_i[:, H:] = 0.0

    def vpack(m):                                               # [145,256] -> [128,2,256]
        out = np.zeros((128, 2, 256))
        out[:, 0, :] = m[:128]
        out[:17, 1, :] = m[128:]
        return out
    cs['wEvr'] = vpack(wEv_r)
    cs['wEvin'] = vpack(-wEv_i)

    rows = np.arange(240)
    pt = ((rows[:, None] // 80) == np.arange(3)[None, :]) / 6400.0   # [240, 3]
    cs['poolT'] = pt.reshape(2, 120, 3).transpose(1, 0, 2)      # [120, 2, 3]
    return cs


def _wshift(W):
    """[16,16,3,3] (o,c,dy,dx) -> [128, 3, 96]: [(c,dy'), dx, (o,r)]."""
    ws = np.zeros((128, 3, 96), NP_DT)
    for c in range(16):
        for o in range(16):
            for r in range(6):
                for dy in range(3):
                    ws[c * 8 + r + dy, :, o * 6 + r] += W[o, c, dy, :]
    return ws


def _wshift_pad(W):
    """[16,16,3,3] -> [128, 3, 128]: [(c,dy'), dx, (o,r)] with zero cols r=6,7."""
    ws = np.zeros((128, 3, 128), NP_DT)
    for c in range(16):
        for o in range(16):
            for r in range(6):
                for dy in range(3):
                    ws[c * 8 + r + dy, :, o * 8 + r] += W[o, c, dy, :]
    return ws


_RAW_CONSTS = _build_consts()
# device dtype per const: bf16 for the F2/denominator path, f32r elsewhere
CONST_BF = {'Fvr', 'Fvi', 'Fvn'}
CONST_DT = {'FuB', 'Eur', 'Eui', 'Eurn', 'wEvr', 'wEvin', 'poolT',
            'E21u', 'E21vr', 'E21vi', 'E21vin', 'E5v', 'E5uc', 'E5us'}
CONSTS = {}
for _k, _v in _RAW_CONSTS.items():
    CONSTS[_k] = np.ascontiguousarray(_v, dtype=NP_DT)


# ---------------------------------------------------------------- bass program
def _dram_ap(handle_ap, offset, dims):
    return bass.AP(tensor=handle_ap.tensor, offset=handle_ap.offset + offset, ap=[list(d) for d in dims])


def _sbuf_ap(t, offset, dims):
    return bass.AP(tensor=t.tensor, offset=t.offset + offset, ap=[list(d) for d in dims])


def build_nc():
    nc = bacc.Bacc("TRN2", target_bir_lowering=False, debug=False)

    x_d = nc.dram_tensor("x", [NF, H, H], DT, kind="ExternalInput").ap()
    ker_d = nc.dram_tensor("ker", [21, 21], DT, kind="ExternalInput").ap()
    wredT_d = nc.dram_tensor("wredT", [64, 16], DT, kind="ExternalInput").ap()
    wg4T_d = nc.dram_tensor("wg4T", [16, 16], F32, kind="ExternalInput").ap()
    wexpT_d = nc.dram_tensor("wexpT", [16, 64], DT, kind="ExternalInput").ap()
    wsh_d = [nc.dram_tensor(f"wsh{i}", [128, 3, 128 if i < 2 else 96], DT,
                            kind="ExternalInput").ap() for i in range(3)]
    cd = {}
    for k, val in CONSTS.items():
        cd[k] = nc.dram_tensor(k, list(val.shape), DT, kind="ExternalInput").ap()
    y_d = nc.dram_tensor("y", [NF, H, H], F32, kind="ExternalOutput").ap()
    dbg = {}
    import os as _os
    if _os.environ.get("KDUMP", "0") == "1":
        for nm, shp, dt in [("d_cls", [16, 248, 246], DT), ("d_h1", [16, 248, 244], DT),
                            ("d_h3", [16, 240, 240], DT), ("d_kp", [16, 9], F32),
                            ("d_Kf2", [96, 3, VH], F32), ("d_rec0", [96, 3, VH], F32),
                            ("d_Cr0", [96, 3, VH], F32), ("d_Ci0", [96, 3, VH], F32),
                            ("d_clear", [16, 246, 246], DT),
                            ("d_Q", [16, 25], DT), ("d_QE0", [5, 290], F32),
                            ("d_Qt", [5, 16, 5], F32), ("d_E5v", [5, 290], F32),
                            ("d_E5uc", [5, 3, 96], F32),
                            ("d_P20", [96, 3, VH], F32)]:
            dbg[nm] = nc.dram_tensor(nm, shp, dt, kind="ExternalOutput").ap()

    with tile.TileContext(nc) as tc:
        _emit(nc, tc, x_d, ker_d, wredT_d, wg4T_d, wexpT_d, wsh_d, cd, y_d, dbg)
    nc.compile()
    return nc


def _emit(nc, tc, x_d, ker_d, wredT_d, wg4T_d, wexpT_d, wsh_d, cd, y_d, dbg={}):
    AF = mybir.ActivationFunctionType
    OP = mybir.AluOpType

    def dump_dram(nm, src_d, nelem):
        if nm not in dbg:
            return
        nc.sync.dma_start(
            bass.AP(tensor=dbg[nm].tensor, offset=dbg[nm].offset, ap=[[1, nelem]]),
            bass.AP(tensor=src_d.tensor, offset=src_d.offset, ap=[[1, nelem]]))

    def dump_sbuf(nm, t):
        if nm not in dbg:
            return
        nc.sync.dma_start(dbg[nm][:], t[:])

    import contextlib
    ctx = contextlib.ExitStack()
    consts = ctx.enter_context(tc.tile_pool(name="consts", bufs=1))
    singles = ctx.enter_context(tc.tile_pool(name="singles", bufs=1))
    ps = ctx.enter_context(tc.tile_pool(name="ps", bufs=8, space="PSUM"))
    dram = ctx.enter_context(tc.tile_pool(name="dram", bufs=1, space="DRAM"))

    _cp = [0]

    def copy_ps(dst, src):
        _cp[0] += 1
        if _cp[0] % 2 == 0:
            nc.vector.tensor_copy(dst, src)
        else:
            nc.scalar.activation(dst, src, AF.Copy)

    # ---- conv-critical consts first (sync/SP queue)
    wredT = consts.tile([64, 16], DT)
    nc.sync.dma_start(wredT[:], wredT_d[:])
    wsh = []
    for i in range(3):
        t = consts.tile([128, 3, 128 if i < 2 else 96], DT, name=f"wsh_sb{i}")
        nc.sync.dma_start(t[:], wsh_d[i][:])
        wsh.append(t)
    kersb = consts.tile([21, 21], DT)
    nc.sync.dma_start(kersb[:], ker_d[:])

    # ---- remaining consts; bf16 ones are uploaded f32 and converted on device
    # (bf16 DRAM uploads corrupt partitions >= 3 through this container's
    # PJRT path, so never DMA bf16 from DRAM)
    cs = {}
    with tc.tile_pool(name="bfstage", bufs=1) as bfstage:
        for k, ap_ in cd.items():
            if k in CONST_BF:
                t = bfstage.tile(list(ap_.shape), ap_.dtype, name=f"c_{k}")
                nc.gpsimd.dma_start(t[:], ap_[:])
                tb = consts.tile(list(ap_.shape), BF, name=f"cb_{k}")
                nc.scalar.activation(tb[:], t[:], AF.Copy)
                cs[k] = tb
            else:
                t = consts.tile(list(ap_.shape), ap_.dtype, name=f"c_{k}")
                nc.gpsimd.dma_start(t[:], ap_[:])
                cs[k] = t
    wg4T = consts.tile([16, 16], F32)
    nc.gpsimd.dma_start(wg4T[:], wg4T_d[:])
    wexpT = consts.tile([16, 64], DT)
    nc.gpsimd.dma_start(wexpT[:], wexpT_d[:])


    # ---- DRAM scratch
    cls_d = dram.tile([16, 248, 246], DT)
    h3_d = dram.tile([16, 240, 240], DT)
    clear_d = dram.tile([16, 246, 246], DT)

    # zero the pad rows of cls (rows 246-247) and h1 (rows 246-247)
    zpad32 = singles.tile([16, 2, 246], F32)
    nc.vector.memset(zpad32[:], 0.0)
    zpad = singles.tile([16, 2, 246], DT)
    nc.scalar.activation(zpad[:], zpad32[:], mybir.ActivationFunctionType.Copy)
    nc.sync.dma_start(_dram_ap(cls_d, 246 * 246, [[248 * 246, 16], [246, 2], [1, 246]]),
                      zpad[:])

    # ---- conv_red: cls[o, p] = sum_c wredT[c, o] * x[c, p]
    x_flat = x_d.rearrange("c h w -> c (h w)")
    cls_flat = cls_d.rearrange("o h w -> o (h w)")

    dump_dram("d_cls", cls_d, 16 * 248 * 246)

    # ---- Kf via E21 (once per core); E21v* padded to 290 free for f32r rate
    T21 = singles.tile([21, 576], DT)
    for nch in range(2):
        pt = ps.tile([21, 288], F32, tag="ps", name="ps_t21")
        nc.tensor.matmul(pt[:], kersb[:], cs['E21u'][:, nch * 288:(nch + 1) * 288],
                         start=True, stop=True)
        nc.scalar.activation(T21[:, nch * 288:(nch + 1) * 288], pt[:], AF.Copy)
    Kfr = singles.tile([96, 3, VH], F32)
    Kfi = singles.tile([96, 3, VH], F32)
    for m3 in range(3):
        ptr = ps.tile([96, 290], F32, tag="ps", name="ps_kfr")
        nc.tensor.matmul(ptr[:], T21[:, m3 * 96:(m3 + 1) * 96], cs['E21vr'][:],
                         start=True, stop=False)
        nc.tensor.matmul(ptr[:], T21[:, 288 + m3 * 96:288 + (m3 + 1) * 96], cs['E21vin'][:],
                         start=False, stop=True)
        nc.scalar.activation(Kfr[:, m3, :], ptr[:, :VH], AF.Copy)
        pti = ps.tile([96, 290], F32, tag="ps", name="ps_kfi")
        nc.tensor.matmul(pti[:], T21[:, m3 * 96:(m3 + 1) * 96], cs['E21vi'][:],
                         start=True, stop=False)
        nc.tensor.matmul(pti[:], T21[:, 288 + m3 * 96:288 + (m3 + 1) * 96], cs['E21vr'][:],
                         start=False, stop=True)
        nc.scalar.activation(Kfi[:, m3, :], pti[:, :VH], AF.Copy)
    Kf2 = singles.tile([96, 3, VH], F32)
    sqt = singles.tile([96, 3, VH], F32)
    nc.scalar.activation(Kf2[:], Kfr[:], AF.Square)
    nc.scalar.activation(sqt[:], Kfi[:], AF.Square)
    nc.vector.tensor_add(Kf2[:], Kf2[:], sqt[:])
    dump_sbuf("d_Kf2", Kf2)

    # ---- 3x3 conv chain: h1/h2 SBUF-resident in gathered rhs layout
    # [(c, rm 0..7), t-slot, x]; rows 6t+rm; rm 6,7 filled by shift DMAs from
    # slot t+1 rm 0,1.  conv1/conv2 use 128-col zero-padded lhsT so PSUM
    # partitions match the rhs layout (partition-identity leaky writes).
    convp_cm = tc.tile_pool(name="convp", bufs=2)
    convp = convp_cm.__enter__()
    hp_cm = tc.tile_pool(name="hpool", bufs=1)
    hp = hp_cm.__enter__()
    h1rhs = hp.tile([128, 42, 244], DT, name="h1rhs")
    h2rhs = hp.tile([128, 42, 242], DT, name="h2rhs")
    z128 = convp.tile([128, 2, 244], F32, tag="z128", bufs=1)
    nc.vector.memset(z128[:], 0.0)
    nc.vector.tensor_copy(h1rhs[:, 40:42, :], z128[:])
    nc.vector.tensor_copy(h2rhs[:, 40:42, :], z128[:, :, :242])

    def shift78(hrhs, psz, W_out, s0, s1):
        nsl = s1 - s0 + 1
        if nsl <= 0:
            return
        for r in range(2):
            nc.gpsimd.dma_start(
                _sbuf_ap(hrhs, (6 + r) * psz + s0 * W_out,
                         [[8 * psz, 16], [W_out, nsl], [1, W_out]]),
                _sbuf_ap(hrhs, r * psz + (s0 + 1) * W_out,
                         [[8 * psz, 16], [W_out, nsl], [1, W_out]]))

    def leaky_out(dst_slice, pt, c2, W_out, tag):
        ab = convp.tile([128, 2, 244], F32, tag="convab")
        nc.scalar.activation(ab[:, :c2, :W_out],
                             pt[:, :c2 * W_out].rearrange("m (t j) -> m t j", t=c2),
                             AF.Abs, scale=0.45)
        nc.vector.scalar_tensor_tensor(
            out=dst_slice, in0=pt[:, :c2 * W_out].rearrange("m (t j) -> m t j", t=c2),
            scalar=0.55, in1=ab[:, :c2, :W_out], op0=OP.mult, op1=OP.add)

    SLAB = 2048
    nslab = (NPIX + SLAB - 1) // SLAB
    RSLAB = 2048
    nrslab = (NPIX + RSLAB - 1) // RSLAB
    redp_cm = tc.tile_pool(name="redp", bufs=2)
    redp = redp_cm.__enter__()

    def emit_red(s):
        j0 = s * RSLAB
        jn = min(RSLAB, NPIX - j0)
        xs = redp.tile([64, RSLAB], DT, tag="xslab")
        nc.sync.dma_start(xs[:, :jn], x_flat[:, j0:j0 + jn])
        clssb = redp.tile([16, RSLAB], DT, tag="clssb")
        for j in range(0, jn, 512):
            w = min(512, jn - j)
            pt = ps.tile([16, 512], F32, tag="ps", name="ps_red")
            nc.tensor.matmul(pt[:, :w], wredT[:], xs[:, j:j + w], start=True, stop=True)
            copy_ps(clssb[:, j:j + w], pt[:, :w])
        nc.gpsimd.dma_start(cls_flat[:, j0:j0 + jn], clssb[:, :jn])

    # conv1: cls (DRAM) -> h1rhs
    GRP1 = 8
    W1i, W1o = 246, 244
    psz1 = 42 * W1o

    rhs_of = {}

    def emit_c1_gather(gi):
        t0 = gi * GRP1
        cnt = min(GRP1, 41 - t0)
        rhs = convp.tile([128, GRP1, W1i], DT, tag="convrhs")
        row_sz = GRP1 * W1i
        for dy in range(8):
            (nc.gpsimd if dy < 2 else nc.sync).dma_start(
                _sbuf_ap(rhs, dy * row_sz, [[8 * row_sz, 16], [W1i, cnt], [1, W1i]]),
                _dram_ap(cls_d, (6 * t0 + dy) * W1i,
                         [[248 * W1i, 16], [6 * W1i, cnt], [1, W1i]]))
        rhs_of[gi] = rhs

    def emit_c1(gi):
        t0 = gi * GRP1
        cnt = min(GRP1, 41 - t0)
        if gi not in rhs_of:
            emit_c1_gather(gi)
        rhs = rhs_of.pop(gi)
        for tp in range(0, cnt, 2):
            c2 = min(2, cnt - tp)
            pt = ps.tile([128, 2 * W1o], F32, tag="ps", name="ps_c1")
            for dx in range(3):
                nc.tensor.matmul(pt[:, :c2 * W1o], wsh[0][:, dx, :],
                                 rhs[:, tp:tp + c2, dx:dx + W1o],
                                 start=(dx == 0), stop=(dx == 2))
            leaky_out(h1rhs[:, t0 + tp:t0 + tp + c2, :], pt, c2, W1o, "convab1")
        shift78(h1rhs, psz1, W1o, max(0, t0 - 1),
                t0 + cnt - 2 if t0 + cnt < 41 else 40)

    # conv2: h1rhs -> h2rhs (no gather DMAs)
    W2o = 242
    psz2 = 42 * W2o

    def emit_c2(gi):
        t0 = gi * GRP1
        cnt = min(GRP1, 41 - t0)
        for tp in range(0, cnt, 2):
            c2 = min(2, cnt - tp)
            g = t0 + tp
            pt = ps.tile([128, 2 * W2o], F32, tag="ps", name="ps_c2")
            for dx in range(3):
                nc.tensor.matmul(pt[:, :c2 * W2o], wsh[1][:, dx, :],
                                 h1rhs[:, g:g + c2, dx:dx + W2o],
                                 start=(dx == 0), stop=(dx == 2))
            leaky_out(h2rhs[:, g:g + c2, :], pt, c2, W2o, "convab2")
        shift78(h2rhs, psz2, W2o, max(0, t0 - 1),
                t0 + cnt - 2 if t0 + cnt < 41 else 40)

    # conv3: h2rhs -> h3_d (DRAM) via out8 staging
    W3o = 240

    def emit_c3(gi):
        t0 = gi * 8
        cnt = min(8, 40 - t0)
        out8 = convp.tile([96, 8, W3o], DT, tag="convrhs")
        for tp in range(0, cnt, 2):
            c2 = min(2, cnt - tp)
            pt = ps.tile([96, 2 * W3o], F32, tag="ps", name="ps_c3")
            for dx in range(3):
                nc.tensor.matmul(pt[:, :c2 * W3o], wsh[2][:, dx, :],
                                 h2rhs[:, t0 + tp:t0 + tp + c2, dx:dx + W3o],
                                 start=(dx == 0), stop=(dx == 2))
            nc.scalar.activation(out8[:, tp:tp + c2, :],
                                 pt[:, :c2 * W3o].rearrange("m (t j) -> m t j", t=c2),
                                 AF.Copy)
        out_sz = 8 * W3o
        for r in range(6):
            q = nc.gpsimd if r < 3 else nc.sync
            q.dma_start(
                _dram_ap(h3_d, (6 * t0 + r) * W3o,
                         [[240 * W3o, 16], [6 * W3o, cnt], [1, W3o]]),
                _sbuf_ap(out8, r * out_sz,
                         [[6 * out_sz, 16], [W3o, cnt], [1, W3o]]))

    for s in range(nrslab):
        emit_red(s)
        if s == 6:
            emit_c1_gather(0)
        elif s == 12:
            emit_c1_gather(1)
    redp_cm.__exit__(None, None, None)
    NG1 = (41 + GRP1 - 1) // GRP1
    for g in range(NG1):
        emit_c1(g)
    clsT_pre = []
    for o in range(2):
        t = consts.tile([123, 2, 246], DT, name=f"clsT_pre{o}")
        nc.sync.dma_start(t[:], _dram_ap(cls_d, o * 248 * 246,
                                         [[246, 123], [123 * 246, 2], [1, 246]]))
        clsT_pre.append(t)
    d3 = 0
    for g in range(NG1):
        emit_c2(g)
        while d3 < 5 and d3 + 1 < g:
            emit_c3(d3)
            d3 += 1
    while d3 < 5:
        emit_c3(d3)
        d3 += 1
    hp_cm.__exit__(None, None, None)

    dump_dram("d_h3", h3_d, 16 * 240 * 240)


    # ---- per-plane FFT: PASS A (F1, F2, numerator A = C * conj(Kf))
    # Runs before the pool/kp/Q chain so that chain overlaps with PE work.
    astore_cm = tc.tile_pool(name="astore", bufs=1)
    astore = astore_cm.__enter__()
    Ar_t = [astore.tile([96, 3, VH], F32, name=f"Ar{o}") for o in range(16)]
    Ain_t = [astore.tile([96, 3, VH], F32, name=f"Ain{o}") for o in range(16)]
    planeA_cm = tc.tile_pool(name="planeA", bufs=3)
    planeA = planeA_cm.__enter__()
    def emit_passA(o):
        if o < 2:
            clsT = clsT_pre[o]
        else:
            clsT = planeA.tile([123, 2, 246], DT, tag="clsT")
            nc.sync.dma_start(clsT[:],
                              _dram_ap(cls_d, o * 248 * 246,
                                       [[246, 123], [123 * 246, 2], [1, 246]]))
        # F1: R1T[w', u] = sum_i cls[i, w'] Fpad[i, u]   (bf16 out for F2)
        R1T = planeA.tile([123, 2, 576], BF, tag="R1T")
        for m in range(2):
            for nch in range(2):
                pt = ps.tile([123, 288], F32, tag="ps", name="ps_f1")
                for k in range(2):
                    nc.tensor.matmul(pt[:], clsT[:, k, m * 123:(m + 1) * 123],
                                     cs['FuB'][:, k, nch * 288:(nch + 1) * 288],
                                     start=(k == 0), stop=(k == 1))
                nc.scalar.activation(R1T[:, m, nch * 288:(nch + 1) * 288], pt[:], AF.Copy)
        # F2 (bf16, half-spectrum): C[u, v] consumed straight from PSUM by the
        # numerator A = C * conj(Kf) (no Cr/Ci SBUF staging).
        tA = planeA.tile([96, 3, VH], F32, tag="tA", bufs=2)
        tB = planeA.tile([96, 3, VH], F32, tag="tB", bufs=2)
        tC = planeA.tile([96, 3, VH], F32, tag="tC", bufs=2)
        tD = planeA.tile([96, 3, VH], F32, tag="tD", bufs=2)
        Ci = planeA.tile([96, 3, VH], F32, tag="Ci", bufs=2)
        for m3 in range(3):
            pcr = ps.tile([96, VH], F32, tag="ps", name="ps_cr")
            for k in range(2):
                nc.tensor.matmul(pcr[:], R1T[:, k, m3 * 96:(m3 + 1) * 96],
                                 cs['Fvr'][:, k, :], start=(k == 0), stop=False)
            for k in range(2):
                nc.tensor.matmul(pcr[:], R1T[:, k, 288 + m3 * 96:288 + (m3 + 1) * 96],
                                 cs['Fvn'][:, k, :], start=False, stop=(k == 1))
            nc.vector.tensor_mul(tA[:, m3, :], pcr[:], Kfr[:, m3, :])
            nc.vector.tensor_mul(tC[:, m3, :], pcr[:], Kfi[:, m3, :])
            pci = ps.tile([96, VH], F32, tag="ps", name="ps_ci")
            for k in range(2):
                nc.tensor.matmul(pci[:], R1T[:, k, m3 * 96:(m3 + 1) * 96],
                                 cs['Fvi'][:, k, :], start=(k == 0), stop=False)
            for k in range(2):
                nc.tensor.matmul(pci[:], R1T[:, k, 288 + m3 * 96:288 + (m3 + 1) * 96],
                                 cs['Fvr'][:, k, :], start=False, stop=(k == 1))
            nc.scalar.activation(Ci[:, m3, :], pci[:], AF.Copy)
            nc.gpsimd.tensor_mul(tB[:, m3, :], Ci[:, m3, :], Kfi[:, m3, :])
            nc.vector.tensor_mul(tD[:, m3, :], Ci[:, m3, :], Kfr[:, m3, :])
        nc.vector.tensor_add(Ar_t[o][:], tA[:], tB[:])
        nc.gpsimd.tensor_tensor(Ain_t[o][:], tC[:], tD[:], mybir.AluOpType.subtract)

    for o in range(11):
        emit_passA(o)

    # ---- adaptive pool -> kp [16, 9]
    P1sb = singles.tile([3, 16, 240], F32)
    for cc in range(8):
        h3t = convp.tile([120, 2, 2, 240], DT, tag="h3t")
        for rc in range(2):
            nc.sync.dma_start(
                h3t[:, rc, :, :],
                _dram_ap(h3_d, cc * 2 * 240 * 240 + rc * 120 * 240,
                         [[240, 120], [240 * 240, 2], [1, 240]]))
        pt = ps.tile([3, 480], F32, tag="ps", name="ps_pool")
        for rc in range(2):
            nc.tensor.matmul(pt[:], cs['poolT'][:, rc, :],
                             h3t[:, rc, :, :].rearrange("p c w -> p (c w)"),
                             start=(rc == 0), stop=(rc == 1))
        nc.scalar.activation(P1sb[:, cc * 2:(cc + 1) * 2, :],
                             pt[:].rearrange("m (c w) -> m c w", c=2), AF.Copy)
    pooled = singles.tile([3, 16, 3], F32)
    nc.vector.tensor_reduce(pooled[:], P1sb[:].rearrange("p c (bx q) -> p c bx q", q=80),
                            axis=mybir.AxisListType.X, op=OP.add)
    pooled_c = singles.tile([16, 9], F32)
    for by in range(3):
        nc.sync.dma_start(pooled_c[:, by * 3:(by + 1) * 3], pooled[by:by + 1, :, :])

    kp = singles.tile([16, 9], F32)
    pt = ps.tile([16, 9], F32, tag="ps", name="ps_kp")
    nc.tensor.matmul(pt[:], wg4T[:], pooled_c[:], start=True, stop=True)
    ekp = singles.tile([16, 9], F32)
    nc.scalar.activation(ekp[:], pt[:], AF.Exp)
    kmean = singles.tile([16, 1], F32)
    nc.vector.tensor_reduce(kmean[:], ekp[:], axis=mybir.AxisListType.X, op=OP.add)
    kmean9 = singles.tile([16, 1], F32)
    nc.scalar.mul(kmean9[:], kmean[:], 1.0 / 9.0)
    nc.vector.tensor_scalar(out=kp[:], in0=ekp[:], scalar1=kmean9[:], scalar2=None,
                            op0=OP.subtract)
    dump_sbuf("d_kp", kp)

    # ---- Q autocorrelation [16, 25] then Qt [5, 16, 5] (bf16)
    Q = singles.tile([16, 25], DT)
    qtmp = singles.tile([16, 9], F32)
    qtmp2 = singles.tile([16, 9], F32)
    kp3 = kp[:].rearrange("o (r c) -> o r c", r=3)
    for dr in range(-2, 3):
        for dc in range(-2, 3):
            r0, r1 = max(0, dr), min(3, 3 + dr)
            c0, c1 = max(0, dc), min(3, 3 + dc)
            nr, ncol = r1 - r0, c1 - c0
            idx = (dr + 2) * 5 + (dc + 2)
            eng = nc.vector if idx % 2 == 0 else nc.gpsimd
            qt2 = qtmp[:, :nr * ncol] if idx % 2 == 0 else qtmp2[:, :nr * ncol]
            eng.tensor_mul(qt2.rearrange("o (r c) -> o r c", r=nr),
                           kp3[:, r0:r1, c0:c1],
                           kp3[:, r0 - dr:r1 - dr, c0 - dc:c1 - dc])
            with nc.allow_low_precision(reason="f32r bits == f32 bits"):
                nc.vector.tensor_reduce(Q[:, idx:idx + 1], qt2,
                                        axis=mybir.AxisListType.X, op=OP.add)
    dump_sbuf("d_Q", Q)
    Qt = singles.tile([5, 16, 5], DT)
    Qv = Q[:].rearrange("o (dr dc) -> o dr dc", dc=5)
    for dc in range(5):
        nc.sync.dma_start(Qt[dc:dc + 1, :, :], Qv[:, :, dc])

    for o in range(11, 16):
        emit_passA(o)
    planeA_cm.__exit__(None, None, None)
    if "d_Qt" in dbg:
        qtb = singles.tile([5, 16, 5], F32, name="qtb")
        nc.scalar.activation(qtb[:], Qt[:], AF.Copy)
        dump_sbuf("d_Qt", qtb)
        e5f = singles.tile([5, 290], F32, name="e5f")
        nc.scalar.activation(e5f[:], cs['E5v'][:], AF.Copy)
        dump_sbuf("d_E5v", e5f)
        e5u = singles.tile([5, 3, 96], F32, name="e5u")
        nc.scalar.activation(e5u[:], cs['E5uc'][:], AF.Copy)
        dump_sbuf("d_E5uc", e5u)


    # ---- per-plane FFT: PASS B (denominator pipelined one plane ahead of IFFT)
    plane_cm = tc.tile_pool(name="plane", bufs=3)
    plane = plane_cm.__enter__()
    rec_of = {}

    def emit_den(o):
        pqe = ps.tile([5, 290], F32, tag="ps", name="ps_qe")
        nc.tensor.matmul(pqe[:], Qt[:, o, :], cs['E5v'][:], start=True, stop=True)
        QE2 = plane.tile([5, 2, 290], DT, tag="QE", bufs=2)
        nc.scalar.activation(QE2[:, 0, :], pqe[:], AF.Copy)
        nc.scalar.activation(QE2[:, 1, 0:VH], pqe[:, VH:290], AF.Copy)
        if o == 0 and "d_QE0" in dbg:
            qef = plane.tile([5, 290], F32, tag="qef", bufs=1)
            nc.scalar.activation(qef[:], QE2[:, 0, :], AF.Copy)
            dump_sbuf("d_QE0", qef)
        rec = plane.tile([96, 3, VH], F32, tag="rec", bufs=2)
        dsb = plane.tile([96, 3, VH], F32, tag="dsb", bufs=2)
        for m3 in range(3):
            pden = ps.tile([96, 290], F32, tag="ps", name="ps_den")
            nc.tensor.matmul(pden[:], cs['E5uc'][:, m3, :], QE2[:, 0, :],
                             start=True, stop=False)
            nc.tensor.matmul(pden[:], cs['E5us'][:, m3, :], QE2[:, 1, :],
                             start=False, stop=True)
            if o == 0 and "d_P20" in dbg:
                p2f = plane.tile([96, 3, VH], F32, tag="p2f", bufs=1, name="p2f")
                nc.scalar.activation(p2f[:, m3, :], pden[:, :VH], AF.Copy)
                if m3 == 2:
                    dump_sbuf("d_P20", p2f)
            nc.vector.tensor_add(dsb[:, m3, :], pden[:, :VH], Kf2[:, m3, :])
            nc.vector.reciprocal_approx_fast(rec[:, m3, :], dsb[:, m3, :])
        if o == 0:
            dump_sbuf("d_rec0", rec)
        rec_of[o] = rec

    def emit_ifft(o):
        rec = rec_of.pop(o)
        # Z = A * rec
        Zr = plane.tile([96, 3, VH], DT, tag="Zr")
        Zin = plane.tile([96, 3, VH], DT, tag="Zin")
        nc.vector.tensor_mul(Zr[:], Ar_t[o][:], rec[:])
        nc.vector.tensor_mul(Zin[:], Ain_t[o][:], rec[:])
        # I1 (4-group): HrT[v, a'], HiT[v, a']
        HrT = plane.tile([128, 2, 256], DT, tag="HrT", bufs=2)
        HiT = plane.tile([128, 2, 256], DT, tag="HiT", bufs=2)
        for m2 in range(2):
            vc = 128 if m2 == 0 else 17
            vs = slice(m2 * 128, m2 * 128 + vc)
            phr = ps.tile([128, 256], F32, tag="ps", name="ps_hr")
            for k in range(3):
                nc.tensor.matmul(phr[:vc], Zr[:, k, vs], cs['Eur'][:, k, :],
                                 start=(k == 0), stop=False)
            for k in range(3):
                nc.tensor.matmul(phr[:vc], Zin[:, k, vs], cs['Eui'][:, k, :],
                                 start=False, stop=(k == 2))
            copy_ps(HrT[:vc, m2, :], phr[:vc])
            phi = ps.tile([128, 256], F32, tag="ps", name="ps_hi")
            for k in range(3):
                nc.tensor.matmul(phi[:vc], Zr[:, k, vs], cs['Eui'][:, k, :],
                                 start=(k == 0), stop=False)
            for k in range(3):
                nc.tensor.matmul(phi[:vc], Zin[:, k, vs], cs['Eurn'][:, k, :],
                                 start=False, stop=(k == 2))
            copy_ps(HiT[:vc, m2, :], phi[:vc])
        # I2: clear[a, b]
        clearsb = plane.tile([123, 2, 246], DT, tag="clearsb")
        for ma in range(2):
            asl = slice(ma * 123, (ma + 1) * 123)
            pcl = ps.tile([123, 256], F32, tag="ps", name="ps_cl")
            nc.tensor.matmul(pcl[:], HrT[:, 0, asl], cs['wEvr'][:, 0, :],
                             start=True, stop=False)
            nc.tensor.matmul(pcl[:], HrT[:17, 1, asl], cs['wEvr'][:17, 1, :],
                             start=False, stop=False)
            nc.tensor.matmul(pcl[:], HiT[:, 0, asl], cs['wEvin'][:, 0, :],
                             start=False, stop=False)
            nc.tensor.matmul(pcl[:], HiT[:17, 1, asl], cs['wEvin'][:17, 1, :],
                             start=False, stop=True)
            copy_ps(clearsb[:, ma, :], pcl[:, 0:246])
        nc.sync.dma_start(
            _dram_ap(clear_d, o * 246 * 246, [[246, 123], [123 * 246, 2], [1, 246]]),
            clearsb[:])

    emit_den(0)
    for o in range(16):
        if o + 1 < 16:
            emit_den(o + 1)
        emit_ifft(o)
    plane_cm.__exit__(None, None, None)
    astore_cm.__exit__(None, None, None)
    convp_cm.__exit__(None, None, None)
    dump_dram("d_clear", clear_d, 16 * 246 * 246)

    # ---- conv_exp: y[e, p] = sum_o wexpT[o, e] * clear[o, p]
    clear_flat = clear_d.rearrange("o h w -> o (h w)")
    y_flat = y_d.rearrange("e h w -> e (h w)")
    with tc.tile_pool(name="expp", bufs=3) as expp:
        for s in range(nslab):
            j0 = s * SLAB
            jn = min(SLAB, NPIX - j0)
            csl = expp.tile([16, SLAB], DT, tag="clearslab")
            nc.sync.dma_start(csl[:, :jn], clear_flat[:, j0:j0 + jn])
            ysb = expp.tile([64, SLAB], F32, tag="ysb")
            for j in range(0, jn, 512):
                w = min(512, jn - j)
                pt = ps.tile([64, 512], F32, tag="ps", name="ps_exp")
                nc.tensor.matmul(pt[:, :w], wexpT[:], csl[:, j:j + w], start=True, stop=True)
                copy_ps(ysb[:, j:j + w], pt[:, :w])
            nc.gpsimd.dma_start(y_flat[:, j0:j0 + jn], ysb[:, :jn])

    ctx.close()


_NC_CACHE = None
_LAST_RESULT = None
TRACE = False


def _get_nc():
    global _NC_CACHE
    if _NC_CACHE is None:
        _NC_CACHE = build_nc()
    return _NC_CACHE


def kernel(**inputs):
    nc = _get_nc()
    x = np.asarray(inputs['x'], np.float32)
    kerf = np.asarray(inputs['kernel'], np.float32)
    w_red = np.asarray(inputs['w_red'], np.float32)[:, :, 0, 0]     # [16, 64]
    w_g = [np.asarray(inputs[f'w_g{i}'], np.float32) for i in (1, 2, 3)]
    w_g4 = np.asarray(inputs['w_g4'], np.float32)[:, :, 0, 0]       # [16, 16]
    w_exp = np.asarray(inputs['w_exp'], np.float32)[:, :, 0, 0]     # [64, 16]

    shared = {
        'wredT': np.ascontiguousarray(w_red.T, NP_DT),
        'wg4T': np.ascontiguousarray(w_g4.T, NP_DT),
        'wexpT': np.ascontiguousarray(w_exp.T, NP_DT),
    }
    shared['wsh0'] = _wshift_pad(w_g[0])
    shared['wsh1'] = _wshift_pad(w_g[1])
    shared['wsh2'] = _wshift(w_g[2])
    for k, val in CONSTS.items():
        shared[k] = val

    in_maps = []
    for b in range(B):
        m = dict(shared)
        m['x'] = np.ascontiguousarray(x[b], NP_DT)
        m['ker'] = np.ascontiguousarray(kerf[b, 0], NP_DT)
        in_maps.append(m)

    global _LAST_RESULT
    res = run_bass_kernel_spmd(nc, in_maps, core_ids=list(range(B)), trace=TRACE)
    _LAST_RESULT = res
    y = np.stack([res.results[b]['y'] for b in range(B)], axis=0)
    return y.astype(np.float32)


# revision 33
# speedup vs baseline: 1.0289x; 1.0289x over previous
"""Trainium2 Bass kernel for nn_CLS_30562987278491 (Wiener-deconvolution net).

Self-contained: hardcodes shapes B=8, NF=64, C=16, H=W=246, ks=21, FFT N=288.
Sharding: data-parallel over batch B across the 8 NeuronCores (1 image/core).

Decomposition (validated stage-by-stage against the jax reference):
  - conv_red (1x1) as matmul over the channel dim (2048-pixel slabs).
  - 3x3 convs via the R=6 row-shift scheme.  conv1 gathers cls from DRAM;
    h1/h2 stay SBUF-RESIDENT in the gathered rhs layout [(c, rm 0..7), t, x]
    using 128-col zero-padded lhsT (PSUM partitions == rhs partitions, so the
    leaky write is partition-identity) + tiny partition-shift DMAs that fill
    rm 6,7 of slot t from rm 0,1 of slot t+1.  No h1/h2 DRAM round-trips.
  - adaptive pool 3x3 via a [240,3] ones-matmul + free-dim reduce.
  - FFT as DFT matmuls: edge-replication pad folded into Fpad [246,288];
    Hermitian half-spectrum (v < 145).  F2 runs in bf16 (1 cyc/row at 145
    free); the Wiener denominator |Pf|^2 comes from the 5x5 autocorrelation Q
    of kernel_P via f32r matmuls at 290 free (packed QE layout), and Kf2 is
    added on DVE (no identity-matmul).  Plane loop is split in two passes:
    pass A (F1/F2/numerator A = C*conj(Kf)) runs while the pool/kp/Q chain
    resolves; pass B (denominator, Z, IFFT, crop) follows.
  - conv_exp (1x1) as matmul over 2048-pixel slabs.

Perf notes (TimelineSim): 663us baseline -> 451us.  DMA_ENGINES is a single
exclusive resource (~360 GB/s, runs <512B pay 2x); SWDGE costs Pool 994ns
fixed per gpsimd DMA; HWDGE 625ns per sync DMA; f32 matmuls are 4 cyc/row,
f32r 1 cyc/row only at free>=256, bf16 1 cyc/row at any width.  bf16 DMA
(upload or SBUF-SBUF) is BROKEN in this container path - convert on device.
"""
import numpy as np
import ml_dtypes

import concourse.bass as bass
import concourse.bacc as bacc
import concourse.mybir as mybir
import concourse.tile as tile
from concourse.bass_utils import run_bass_kernel_spmd

F32 = mybir.dt.float32
DT = mybir.dt.float32r          # 4-byte, bit-compatible with f32; PE 1 cyc/row at free>=256
BF = mybir.dt.bfloat16
NP_DT = np.float32
NP_BF = ml_dtypes.bfloat16

B, NF, C, H = 8, 64, 16, 246
N = 288
VH = 145                    # N//2 + 1
KS = 21
NPIX = H * H                # 60516
CROP = 21


# ---------------------------------------------------------------- host consts
def _build_consts():
    cs = {}
    u = np.arange(N)
    v = np.arange(VH)
    F = np.exp(-2j * np.pi * np.outer(np.arange(N), u) / N)
    Fpad = np.zeros((H, N), complex)
    Fpad[0] = F[0:22].sum(0)
    Fpad[1:245] = F[22:266]
    Fpad[245] = F[266:288].sum(0)

    FuB = np.concatenate([Fpad.real, Fpad.imag], axis=1)        # [246, 576]
    cs['FuB'] = FuB.reshape(2, 123, 576).transpose(1, 0, 2)     # [123, 2, 576]

    def vchunk(m):                                              # [246,145] -> [123,2,145]
        return m.reshape(2, 123, VH).transpose(1, 0, 2)
    cs['Fvr'] = vchunk(Fpad[:, :VH].real)
    cs['Fvi'] = vchunk(Fpad[:, :VH].imag)
    cs['Fvn'] = vchunk(-Fpad[:, :VH].imag)

    d5 = np.arange(5) - 2
    E5v = np.exp(-2j * np.pi * np.outer(d5, v) / N)             # [5, 145]
    cs['E5v'] = np.concatenate([E5v.real, E5v.imag], axis=1)    # [5, 290]
    th5 = 2 * np.pi * np.outer(d5, u) / N                       # [5, 288]
    cs['E5uc'] = np.cos(th5).reshape(5, 3, 96)
    cs['E5us'] = np.sin(th5).reshape(5, 3, 96)

    d21 = np.arange(21) - 10
    E21u = np.exp(-2j * np.pi * np.outer(d21, u) / N)           # [21, 288]
    cs['E21u'] = np.concatenate([E21u.real, E21u.imag], axis=1)  # [21, 576]
    E21v = np.exp(-2j * np.pi * np.outer(d21, v) / N)           # [21, 145]
    z = np.zeros((21, 290))
    z[:, :VH] = E21v.real
    cs['E21vr'] = z.copy()
    z = np.zeros((21, 290))
    z[:, :VH] = E21v.imag
    cs['E21vi'] = z.copy()
    z = np.zeros((21, 290))
    z[:, :VH] = -E21v.imag
    cs['E21vin'] = z.copy()

    a = CROP + np.arange(256)
    thu = 2 * np.pi * np.outer(u, a) / N                        # [288, 256]
    cs['Eur'] = np.cos(thu).reshape(3, 96, 256).transpose(1, 0, 2)   # [96, 3, 256]
    cs['Eui'] = np.sin(thu).reshape(3, 96, 256).transpose(1, 0, 2)
    cs['Eurn'] = -cs['Eur']

    wv = np.where((v == 0) | (v == N // 2), 1.0, 2.0) / (N * N)
    bb = CROP + np.arange(256)
    thv = 2 * np.pi * np.outer(v, bb) / N                       # [145, 256]
    wEv_r = wv[:, None] * np.cos(thv)
    wEv_i = wv[:, None] * np.sin(thv)
    wEv_r[:, H:] = 0.0
    wEv_i[:, H:] = 0.0

    def vpack(m):                                               # [145,256] -> [128,2,256]
        out = np.zeros((128, 2, 256))
        out[:, 0, :] = m[:128]
        out[:17, 1, :] = m[128:]
        return out
    cs['wEvr'] = vpack(wEv_r)
    cs['wEvin'] = vpack(-wEv_i)

    rows = np.arange(240)
    pt = ((rows[:, None] // 80) == np.arange(3)[None, :]) / 6400.0   # [240, 3]
    cs['poolT'] = pt.reshape(2, 120, 3).transpose(1, 0, 2)      # [120, 2, 3]
    return cs


def _wshift(W):
    """[16,16,3,3] (o,c,dy,dx) -> [128, 3, 96]: [(c,dy'), dx, (o,r)]."""
    ws = np.zeros((128, 3, 96), NP_DT)
    for c in range(16):
        for o in range(16):
            for r in range(6):
                for dy in range(3):
                    ws[c * 8 + r + dy, :, o * 6 + r] += W[o, c, dy, :]
    return ws


def _wshift_pad(W):
    """[16,16,3,3] -> [128, 3, 128]: [(c,dy'), dx, (o,r)] with zero cols r=6,7."""
    ws = np.zeros((128, 3, 128), NP_DT)
    for c in range(16):
        for o in range(16):
            for r in range(6):
                for dy in range(3):
                    ws[c * 8 + r + dy, :, o * 8 + r] += W[o, c, dy, :]
    return ws


_RAW_CONSTS = _build_consts()
# device dtype per const: bf16 for the F2/denominator path, f32r elsewhere
CONST_BF = {'Fvr', 'Fvi', 'Fvn'}
CONST_DT = {'FuB', 'Eur', 'Eui', 'Eurn', 'wEvr', 'wEvin', 'poolT',
            'E21u', 'E21vr', 'E21vi', 'E21vin', 'E5v', 'E5uc', 'E5us'}
CONSTS = {}
for _k, _v in _RAW_CONSTS.items():
    CONSTS[_k] = np.ascontiguousarray(_v, dtype=NP_DT)


# ---------------------------------------------------------------- bass program
def _dram_ap(handle_ap, offset, dims):
    return bass.AP(tensor=handle_ap.tensor, offset=handle_ap.offset + offset, ap=[list(d) for d in dims])


def _sbuf_ap(t, offset, dims):
    return bass.AP(tensor=t.tensor, offset=t.offset + offset, ap=[list(d) for d in dims])


def build_nc():
    nc = bacc.Bacc("TRN2", target_bir_lowering=False, debug=False)

    x_d = nc.dram_tensor("x", [NF, H, H], DT, kind="ExternalInput").ap()
    ker_d = nc.dram_tensor("ker", [21, 21], DT, kind="ExternalInput").ap()
    wredT_d = nc.dram_tensor("wredT", [64, 16], DT, kind="ExternalInput").ap()
    wg4T_d = nc.dram_tensor("wg4T", [16, 16], F32, kind="ExternalInput").ap()
    wexpT_d = nc.dram_tensor("wexpT", [16, 64], DT, kind="ExternalInput").ap()
    wsh_d = [nc.dram_tensor(f"wsh{i}", [128, 3, 128 if i < 2 else 96], DT,
                            kind="ExternalInput").ap() for i in range(3)]
    cd = {}
    for k, val in CONSTS.items():
        cd[k] = nc.dram_tensor(k, list(val.shape), DT, kind="ExternalInput").ap()
    y_d = nc.dram_tensor("y", [NF, H, H], F32, kind="ExternalOutput").ap()
    dbg = {}
    import os as _os
    if _os.environ.get("KDUMP", "0") == "1":
        for nm, shp, dt in [("d_cls", [16, 248, 246], DT), ("d_h1", [16, 248, 244], DT),
                            ("d_h3", [16, 240, 240], DT), ("d_kp", [16, 9], F32),
                            ("d_Kf2", [96, 3, VH], F32), ("d_rec0", [96, 3, VH], F32),
                            ("d_Cr0", [96, 3, VH], F32), ("d_Ci0", [96, 3, VH], F32),
                            ("d_clear", [16, 246, 246], DT),
                            ("d_Q", [16, 25], DT), ("d_QE0", [5, 290], F32),
                            ("d_Qt", [5, 16, 5], F32), ("d_E5v", [5, 290], F32),
                            ("d_E5uc", [5, 3, 96], F32),
                            ("d_P20", [96, 3, VH], F32)]:
            dbg[nm] = nc.dram_tensor(nm, shp, dt, kind="ExternalOutput").ap()

    with tile.TileContext(nc) as tc:
        _emit(nc, tc, x_d, ker_d, wredT_d, wg4T_d, wexpT_d, wsh_d, cd, y_d, dbg)
    nc.compile()
    return nc


def _emit(nc, tc, x_d, ker_d, wredT_d, wg4T_d, wexpT_d, wsh_d, cd, y_d, dbg={}):
    AF = mybir.ActivationFunctionType
    OP = mybir.AluOpType

    def dump_dram(nm, src_d, nelem):
        if nm not in dbg:
            return
        nc.sync.dma_start(
            bass.AP(tensor=dbg[nm].tensor, offset=dbg[nm].offset, ap=[[1, nelem]]),
            bass.AP(tensor=src_d.tensor, offset=src_d.offset, ap=[[1, nelem]]))

    def dump_sbuf(nm, t):
        if nm not in dbg:
            return
        nc.sync.dma_start(dbg[nm][:], t[:])

    import contextlib
    ctx = contextlib.ExitStack()
    consts = ctx.enter_context(tc.tile_pool(name="consts", bufs=1))
    singles = ctx.enter_context(tc.tile_pool(name="singles", bufs=1))
    ps = ctx.enter_context(tc.tile_pool(name="ps", bufs=8, space="PSUM"))
    dram = ctx.enter_context(tc.tile_pool(name="dram", bufs=1, space="DRAM"))

    _cp = [0]

    def copy_ps(dst, src):
        _cp[0] += 1
        if _cp[0] % 2 == 0:
            nc.vector.tensor_copy(dst, src)
        else:
            nc.scalar.activation(dst, src, AF.Copy)

    # ---- conv-critical consts first (sync/SP queue)
    wredT = consts.tile([64, 16], DT)
    nc.sync.dma_start(wredT[:], wredT_d[:])
    wsh = []
    for i in range(3):
        t = consts.tile([128, 3, 128 if i < 2 else 96], DT, name=f"wsh_sb{i}")
        nc.sync.dma_start(t[:], wsh_d[i][:])
        wsh.append(t)
    kersb = consts.tile([21, 21], DT)
    nc.sync.dma_start(kersb[:], ker_d[:])

    # ---- remaining consts; bf16 ones are uploaded f32 and converted on device
    # (bf16 DRAM uploads corrupt partitions >= 3 through this container's
    # PJRT path, so never DMA bf16 from DRAM)
    cs = {}
    with tc.tile_pool(name="bfstage", bufs=1) as bfstage:
        for k, ap_ in cd.items():
            if k in CONST_BF:
                t = bfstage.tile(list(ap_.shape), ap_.dtype, name=f"c_{k}")
                nc.gpsimd.dma_start(t[:], ap_[:])
                tb = consts.tile(list(ap_.shape), BF, name=f"cb_{k}")
                nc.scalar.activation(tb[:], t[:], AF.Copy)
                cs[k] = tb
            else:
                t = consts.tile(list(ap_.shape), ap_.dtype, name=f"c_{k}")
                nc.gpsimd.dma_start(t[:], ap_[:])
                cs[k] = t
    wg4T = consts.tile([16, 16], F32)
    nc.gpsimd.dma_start(wg4T[:], wg4T_d[:])
    wexpT = consts.tile([16, 64], DT)
    nc.gpsimd.dma_start(wexpT[:], wexpT_d[:])


    # ---- DRAM scratch
    cls_d = dram.tile([16, 248, 246], DT)
    h3_d = dram.tile([16, 240, 240], DT)
    clear_d = dram.tile([16, 246, 246], DT)

    # zero the pad rows of cls (rows 246-247) and h1 (rows 246-247)
    zpad32 = singles.tile([16, 2, 246], F32)
    nc.vector.memset(zpad32[:], 0.0)
    zpad = singles.tile([16, 2, 246], DT)
    nc.scalar.activation(zpad[:], zpad32[:], mybir.ActivationFunctionType.Copy)
    nc.sync.dma_start(_dram_ap(cls_d, 246 * 246, [[248 * 246, 16], [246, 2], [1, 246]]),
                      zpad[:])

    # ---- conv_red: cls[o, p] = sum_c wredT[c, o] * x[c, p]
    x_flat = x_d.rearrange("c h w -> c (h w)")
    cls_flat = cls_d.rearrange("o h w -> o (h w)")

    dump_dram("d_cls", cls_d, 16 * 248 * 246)

    # ---- Kf via E21 (once per core); E21v* padded to 290 free for f32r rate
    T21 = singles.tile([21, 576], DT)
    for nch in range(2):
        pt = ps.tile([21, 288], F32, tag="ps", name="ps_t21")
        nc.tensor.matmul(pt[:], kersb[:], cs['E21u'][:, nch * 288:(nch + 1) * 288],
                         start=True, stop=True)
        nc.scalar.activation(T21[:, nch * 288:(nch + 1) * 288], pt[:], AF.Copy)
    Kfr = singles.tile([96, 3, VH], F32)
    Kfi = singles.tile([96, 3, VH], F32)
    for m3 in range(3):
        ptr = ps.tile([96, 290], F32, tag="ps", name="ps_kfr")
        nc.tensor.matmul(ptr[:], T21[:, m3 * 96:(m3 + 1) * 96], cs['E21vr'][:],
                         start=True, stop=False)
        nc.tensor.matmul(ptr[:], T21[:, 288 + m3 * 96:288 + (m3 + 1) * 96], cs['E21vin'][:],
                         start=False, stop=True)
        nc.scalar.activation(Kfr[:, m3, :], ptr[:, :VH], AF.Copy)
        pti = ps.tile([96, 290], F32, tag="ps", name="ps_kfi")
        nc.tensor.matmul(pti[:], T21[:, m3 * 96:(m3 + 1) * 96], cs['E21vi'][:],
                         start=True, stop=False)
        nc.tensor.matmul(pti[:], T21[:, 288 + m3 * 96:288 + (m3 + 1) * 96], cs['E21vr'][:],
                         start=False, stop=True)
        nc.scalar.activation(Kfi[:, m3, :], pti[:, :VH], AF.Copy)
    Kf2 = singles.tile([96, 3, VH], F32)
    sqt = singles.tile([96, 3, VH], F32)
    nc.scalar.activation(Kf2[:], Kfr[:], AF.Square)
    nc.scalar.activation(sqt[:], Kfi[:], AF.Square)
    nc.vector.tensor_add(Kf2[:], Kf2[:], sqt[:])
    dump_sbuf("d_Kf2", Kf2)

    # ---- 3x3 conv chain: h1/h2 SBUF-resident in gathered rhs layout
    # [(c, rm 0..7), t-slot, x]; rows 6t+rm; rm 6,7 filled by shift DMAs from
    # slot t+1 rm 0,1.  conv1/conv2 use 128-col zero-padded lhsT so PSUM
    # partitions match the rhs layout (partition-identity leaky writes).
    convp_cm = tc.tile_pool(name="convp", bufs=2)
    convp = convp_cm.__enter__()
    hp_cm = tc.tile_pool(name="hpool", bufs=1)
    hp = hp_cm.__enter__()
    h1rhs = hp.tile([128, 42, 244], DT, name="h1rhs")
    h2rhs = hp.tile([128, 42, 242], DT, name="h2rhs")
    z128 = convp.tile([128, 2, 244], F32, tag="z128", bufs=1)
    nc.vector.memset(z128[:], 0.0)
    nc.vector.tensor_copy(h1rhs[:, 40:42, :], z128[:])
    nc.vector.tensor_copy(h2rhs[:, 40:42, :], z128[:, :, :242])

    def shift78(hrhs, psz, W_out, s0, s1):
        nsl = s1 - s0 + 1
        if nsl <= 0:
            return
        for r in range(2):
            nc.gpsimd.dma_start(
                _sbuf_ap(hrhs, (6 + r) * psz + s0 * W_out,
                         [[8 * psz, 16], [W_out, nsl], [1, W_out]]),
                _sbuf_ap(hrhs, r * psz + (s0 + 1) * W_out,
                         [[8 * psz, 16], [W_out, nsl], [1, W_out]]))

    def leaky_out(dst_slice, pt, c2, W_out, tag):
        ab = convp.tile([128, 2, 244], F32, tag="convab")
        nc.scalar.activation(ab[:, :c2, :W_out],
                             pt[:, :c2 * W_out].rearrange("m (t j) -> m t j", t=c2),
                             AF.Abs, scale=0.45)
        nc.vector.scalar_tensor_tensor(
            out=dst_slice, in0=pt[:, :c2 * W_out].rearrange("m (t j) -> m t j", t=c2),
            scalar=0.55, in1=ab[:, :c2, :W_out], op0=OP.mult, op1=OP.add)

    SLAB = 2048
    nslab = (NPIX + SLAB - 1) // SLAB
    RSLAB = 2048
    nrslab = (NPIX + RSLAB - 1) // RSLAB
    redp_cm = tc.tile_pool(name="redp", bufs=2)
    redp = redp_cm.__enter__()

    def emit_red(s):
        j0 = s * RSLAB
        jn = min(RSLAB, NPIX - j0)
        xs = redp.tile([64, RSLAB], DT, tag="xslab")
        nc.sync.dma_start(xs[:, :jn], x_flat[:, j0:j0 + jn])
        clssb = redp.tile([16, RSLAB], DT, tag="clssb")
        for j in range(0, jn, 512):
            w = min(512, jn - j)
            pt = ps.tile([16, 512], F32, tag="ps", name="ps_red")
            nc.tensor.matmul(pt[:, :w], wredT[:], xs[:, j:j + w], start=True, stop=True)
            copy_ps(clssb[:, j:j + w], pt[:, :w])
        nc.gpsimd.dma_start(cls_flat[:, j0:j0 + jn], clssb[:, :jn])

    # conv1: cls (DRAM) -> h1rhs
    GRP1 = 8
    W1i, W1o = 246, 244
    psz1 = 42 * W1o

    rhs_of = {}

    def emit_c1_gather(gi):
        t0 = gi * GRP1
        cnt = min(GRP1, 41 - t0)
        rhs = convp.tile([128, GRP1, W1i], DT, tag="convrhs")
        row_sz = GRP1 * W1i
        for dy in range(8):
            (nc.gpsimd if dy < 2 else nc.sync).dma_start(
                _sbuf_ap(rhs, dy * row_sz, [[8 * row_sz, 16], [W1i, cnt], [1, W1i]]),
                _dram_ap(cls_d, (6 * t0 + dy) * W1i,
                         [[248 * W1i, 16], [6 * W1i, cnt], [1, W1i]]))
        rhs_of[gi] = rhs

    def emit_c1(gi):
        t0 = gi * GRP1
        cnt = min(GRP1, 41 - t0)
        if gi not in rhs_of:
            emit_c1_gather(gi)
        rhs = rhs_of.pop(gi)
        for tp in range(0, cnt, 2):
            c2 = min(2, cnt - tp)
            pt = ps.tile([128, 2 * W1o], F32, tag="ps", name="ps_c1")
            for dx in range(3):
                nc.tensor.matmul(pt[:, :c2 * W1o], wsh[0][:, dx, :],
                                 rhs[:, tp:tp + c2, dx:dx + W1o],
                                 start=(dx == 0), stop=(dx == 2))
            leaky_out(h1rhs[:, t0 + tp:t0 + tp + c2, :], pt, c2, W1o, "convab1")
        shift78(h1rhs, psz1, W1o, max(0, t0 - 1),
                t0 + cnt - 2 if t0 + cnt < 41 else 40)

    # conv2: h1rhs -> h2rhs (no gather DMAs)
    W2o = 242
    psz2 = 42 * W2o

    def emit_c2(gi):
        t0 = gi * GRP1
        cnt = min(GRP1, 41 - t0)
        for tp in range(0, cnt, 2):
            c2 = min(2, cnt - tp)
            g = t0 + tp
            pt = ps.tile([128, 2 * W2o], F32, tag="ps", name="ps_c2")
            for dx in range(3):
                nc.tensor.matmul(pt[:, :c2 * W2o], wsh[1][:, dx, :],
                                 h1rhs[:, g:g + c2, dx:dx + W2o],
                                 start=(dx == 0), stop=(dx == 2))
            leaky_out(h2rhs[:, g:g + c2, :], pt, c2, W2o, "convab2")
        shift78(h2rhs, psz2, W2o, max(0, t0 - 1),
                t0 + cnt - 2 if t0 + cnt < 41 else 40)

    # conv3: h2rhs -> h3_d (DRAM) via out8 staging
    W3o = 240

    def emit_c3(gi):
        t0 = gi * 8
        cnt = min(8, 40 - t0)
        out8 = convp.tile([96, 8, W3o], DT, tag="convrhs")
        for tp in range(0, cnt, 2):
            c2 = min(2, cnt - tp)
            pt = ps.tile([96, 2 * W3o], F32, tag="ps", name="ps_c3")
            for dx in range(3):
                nc.tensor.matmul(pt[:, :c2 * W3o], wsh[2][:, dx, :],
                                 h2rhs[:, t0 + tp:t0 + tp + c2, dx:dx + W3o],
                                 start=(dx == 0), stop=(dx == 2))
            nc.scalar.activation(out8[:, tp:tp + c2, :],
                                 pt[:, :c2 * W3o].rearrange("m (t j) -> m t j", t=c2),
                                 AF.Copy)
        out_sz = 8 * W3o
        for r in range(6):
            q = nc.gpsimd if r < 3 else nc.sync
            q.dma_start(
                _dram_ap(h3_d, (6 * t0 + r) * W3o,
                         [[240 * W3o, 16], [6 * W3o, cnt], [1, W3o]]),
                _sbuf_ap(out8, r * out_sz,
                         [[6 * out_sz, 16], [W3o, cnt], [1, W3o]]))

    for s in range(nrslab):
        emit_red(s)
    redp_cm.__exit__(None, None, None)
    NG1 = (41 + GRP1 - 1) // GRP1
    for g in range(NG1):
        emit_c1(g)
    clsT_pre = []
    for o in range(2):
        t = consts.tile([123, 2, 246], DT, name=f"clsT_pre{o}")
        nc.sync.dma_start(t[:], _dram_ap(cls_d, o * 248 * 246,
                                         [[246, 123], [123 * 246, 2], [1, 246]]))
        clsT_pre.append(t)
    d3 = 0
    for g in range(NG1):
        emit_c2(g)
        while d3 < 5 and d3 + 1 < g:
            emit_c3(d3)
            d3 += 1
    while d3 < 5:
        emit_c3(d3)
        d3 += 1
    hp_cm.__exit__(None, None, None)

    dump_dram("d_h3", h3_d, 16 * 240 * 240)


    # ---- per-plane FFT: PASS A (F1, F2, numerator A = C * conj(Kf))
    # Runs before the pool/kp/Q chain so that chain overlaps with PE work.
    astore_cm = tc.tile_pool(name="astore", bufs=1)
    astore = astore_cm.__enter__()
    Ar_t = [astore.tile([96, 3, VH], F32, name=f"Ar{o}") for o in range(16)]
    Ain_t = [astore.tile([96, 3, VH], F32, name=f"Ain{o}") for o in range(16)]
    planeA_cm = tc.tile_pool(name="planeA", bufs=3)
    planeA = planeA_cm.__enter__()
    def emit_passA(o):
        if o < 2:
            clsT = clsT_pre[o]
        else:
            clsT = planeA.tile([123, 2, 246], DT, tag="clsT")
            nc.sync.dma_start(clsT[:],
                              _dram_ap(cls_d, o * 248 * 246,
                                       [[246, 123], [123 * 246, 2], [1, 246]]))
        # F1: R1T[w', u] = sum_i cls[i, w'] Fpad[i, u]   (bf16 out for F2)
        R1T = planeA.tile([123, 2, 576], BF, tag="R1T")
        for m in range(2):
            for nch in range(2):
                pt = ps.tile([123, 288], F32, tag="ps", name="ps_f1")
                for k in range(2):
                    nc.tensor.matmul(pt[:], clsT[:, k, m * 123:(m + 1) * 123],
                                     cs['FuB'][:, k, nch * 288:(nch + 1) * 288],
                                     start=(k == 0), stop=(k == 1))
                nc.scalar.activation(R1T[:, m, nch * 288:(nch + 1) * 288], pt[:], AF.Copy)
        # F2 (bf16, half-spectrum): C[u, v] consumed straight from PSUM by the
        # numerator A = C * conj(Kf) (no Cr/Ci SBUF staging).
        tA = planeA.tile([96, 3, VH], F32, tag="tA", bufs=2)
        tB = planeA.tile([96, 3, VH], F32, tag="tB", bufs=2)
        tC = planeA.tile([96, 3, VH], F32, tag="tC", bufs=2)
        tD = planeA.tile([96, 3, VH], F32, tag="tD", bufs=2)
        Ci = planeA.tile([96, 3, VH], F32, tag="Ci", bufs=2)
        for m3 in range(3):
            pcr = ps.tile([96, VH], F32, tag="ps", name="ps_cr")
            for k in range(2):
                nc.tensor.matmul(pcr[:], R1T[:, k, m3 * 96:(m3 + 1) * 96],
                                 cs['Fvr'][:, k, :], start=(k == 0), stop=False)
            for k in range(2):
                nc.tensor.matmul(pcr[:], R1T[:, k, 288 + m3 * 96:288 + (m3 + 1) * 96],
                                 cs['Fvn'][:, k, :], start=False, stop=(k == 1))
            nc.vector.tensor_mul(tA[:, m3, :], pcr[:], Kfr[:, m3, :])
            nc.vector.tensor_mul(tC[:, m3, :], pcr[:], Kfi[:, m3, :])
            pci = ps.tile([96, VH], F32, tag="ps", name="ps_ci")
            for k in range(2):
                nc.tensor.matmul(pci[:], R1T[:, k, m3 * 96:(m3 + 1) * 96],
                                 cs['Fvi'][:, k, :], start=(k == 0), stop=False)
            for k in range(2):
                nc.tensor.matmul(pci[:], R1T[:, k, 288 + m3 * 96:288 + (m3 + 1) * 96],
                                 cs['Fvr'][:, k, :], start=False, stop=(k == 1))
            nc.scalar.activation(Ci[:, m3, :], pci[:], AF.Copy)
            nc.gpsimd.tensor_mul(tB[:, m3, :], Ci[:, m3, :], Kfi[:, m3, :])
            nc.vector.tensor_mul(tD[:, m3, :], Ci[:, m3, :], Kfr[:, m3, :])
        nc.vector.tensor_add(Ar_t[o][:], tA[:], tB[:])
        nc.gpsimd.tensor_tensor(Ain_t[o][:], tC[:], tD[:], mybir.AluOpType.subtract)

    for o in range(11):
        emit_passA(o)

    # ---- adaptive pool -> kp [16, 9]
    P1sb = singles.tile([3, 16, 240], F32)
    for cc in range(8):
        h3t = convp.tile([120, 2, 2, 240], DT, tag="h3t")
        for rc in range(2):
            nc.sync.dma_start(
                h3t[:, rc, :, :],
                _dram_ap(h3_d, cc * 2 * 240 * 240 + rc * 120 * 240,
                         [[240, 120], [240 * 240, 2], [1, 240]]))
        pt = ps.tile([3, 480], F32, tag="ps", name="ps_pool")
        for rc in range(2):
            nc.tensor.matmul(pt[:], cs['poolT'][:, rc, :],
                             h3t[:, rc, :, :].rearrange("p c w -> p (c w)"),
                             start=(rc == 0), stop=(rc == 1))
        nc.scalar.activation(P1sb[:, cc * 2:(cc + 1) * 2, :],
                             pt[:].rearrange("m (c w) -> m c w", c=2), AF.Copy)
    pooled = singles.tile([3, 16, 3], F32)
    nc.vector.tensor_reduce(pooled[:], P1sb[:].rearrange("p c (bx q) -> p c bx q", q=80),
                            axis=mybir.AxisListType.X, op=OP.add)
    pooled_c = singles.tile([16, 9], F32)
    for by in range(3):
        nc.sync.dma_start(pooled_c[:, by * 3:(by + 1) * 3], pooled[by:by + 1, :, :])

    kp = singles.tile([16, 9], F32)
    pt = ps.tile([16, 9], F32, tag="ps", name="ps_kp")
    nc.tensor.matmul(pt[:], wg4T[:], pooled_c[:], start=True, stop=True)
    ekp = singles.tile([16, 9], F32)
    nc.scalar.activation(ekp[:], pt[:], AF.Exp)
    kmean = singles.tile([16, 1], F32)
    nc.vector.tensor_reduce(kmean[:], ekp[:], axis=mybir.AxisListType.X, op=OP.add)
    kmean9 = singles.tile([16, 1], F32)
    nc.scalar.mul(kmean9[:], kmean[:], 1.0 / 9.0)
    nc.vector.tensor_scalar(out=kp[:], in0=ekp[:], scalar1=kmean9[:], scalar2=None,
                            op0=OP.subtract)
    dump_sbuf("d_kp", kp)

    # ---- Q autocorrelation [16, 25] then Qt [5, 16, 5] (bf16)
    Q = singles.tile([16, 25], DT)
    qtmp = singles.tile([16, 9], F32)
    qtmp2 = singles.tile([16, 9], F32)
    kp3 = kp[:].rearrange("o (r c) -> o r c", r=3)
    for dr in range(-2, 3):
        for dc in range(-2, 3):
            r0, r1 = max(0, dr), min(3, 3 + dr)
            c0, c1 = max(0, dc), min(3, 3 + dc)
            nr, ncol = r1 - r0, c1 - c0
            idx = (dr + 2) * 5 + (dc + 2)
            eng = nc.vector if idx % 2 == 0 else nc.gpsimd
            qt2 = qtmp[:, :nr * ncol] if idx % 2 == 0 else qtmp2[:, :nr * ncol]
            eng.tensor_mul(qt2.rearrange("o (r c) -> o r c", r=nr),
                           kp3[:, r0:r1, c0:c1],
                           kp3[:, r0 - dr:r1 - dr, c0 - dc:c1 - dc])
            with nc.allow_low_precision(reason="f32r bits == f32 bits"):
                nc.vector.tensor_reduce(Q[:, idx:idx + 1], qt2,
                                        axis=mybir.AxisListType.X, op=OP.add)
    dump_sbuf("d_Q", Q)
    Qt = singles.tile([5, 16, 5], DT)
    Qv = Q[:].rearrange("o (dr dc) -> o dr dc", dc=5)
    for dc in range(5):
        nc.sync.dma_start(Qt[dc:dc + 1, :, :], Qv[:, :, dc])

    for o in range(11, 16):
        emit_passA(o)
    planeA_cm.__exit__(None, None, None)
    if "d_Qt" in dbg:
        qtb = singles.tile([5, 16, 5], F32, name="qtb")
        nc.scalar.activation(qtb[:], Qt[:], AF.Copy)
        dump_sbuf("d_Qt", qtb)
        e5f = singles.tile([5, 290], F32, name="e5f")
        nc.scalar.activation(e5f[:], cs['E5v'][:], AF.Copy)
        dump_sbuf("d_E5v", e5f)
        e5u = singles.tile([5, 3, 96], F32, name="e5u")
        nc.scalar.activation(e5u[:], cs['E5uc'][:], AF.Copy)
        dump_sbuf("d_E5uc", e5u)


    # ---- per-plane FFT: PASS B (denominator pipelined one plane ahead of IFFT)
    plane_cm = tc.tile_pool(name="plane", bufs=3)
    plane = plane_cm.__enter__()
    rec_of = {}

    def emit_den(o):
        pqe = ps.tile([5, 290], F32, tag="ps", name="ps_qe")
        nc.tensor.matmul(pqe[:], Qt[:, o, :], cs['E5v'][:], start=True, stop=True)
        QE2 = plane.tile([5, 2, 290], DT, tag="QE", bufs=2)
        nc.scalar.activation(QE2[:, 0, :], pqe[:], AF.Copy)
        nc.scalar.activation(QE2[:, 1, 0:VH], pqe[:, VH:290], AF.Copy)
        if o == 0 and "d_QE0" in dbg:
            qef = plane.tile([5, 290], F32, tag="qef", bufs=1)
            nc.scalar.activation(qef[:], QE2[:, 0, :], AF.Copy)
            dump_sbuf("d_QE0", qef)
        rec = plane.tile([96, 3, VH], F32, tag="rec", bufs=2)
        dsb = plane.tile([96, 3, VH], F32, tag="dsb", bufs=2)
        for m3 in range(3):
            pden = ps.tile([96, 290], F32, tag="ps", name="ps_den")
            nc.tensor.matmul(pden[:], cs['E5uc'][:, m3, :], QE2[:, 0, :],
                             start=True, stop=False)
            nc.tensor.matmul(pden[:], cs['E5us'][:, m3, :], QE2[:, 1, :],
                             start=False, stop=True)
            if o == 0 and "d_P20" in dbg:
                p2f = plane.tile([96, 3, VH], F32, tag="p2f", bufs=1, name="p2f")
                nc.scalar.activation(p2f[:, m3, :], pden[:, :VH], AF.Copy)
                if m3 == 2:
                    dump_sbuf("d_P20", p2f)
            nc.vector.tensor_add(dsb[:, m3, :], pden[:, :VH], Kf2[:, m3, :])
            nc.vector.reciprocal_approx_fast(rec[:, m3, :], dsb[:, m3, :])
        if o == 0:
            dump_sbuf("d_rec0", rec)
        rec_of[o] = rec

    def emit_ifft(o):
        rec = rec_of.pop(o)
        # Z = A * rec
        Zr = plane.tile([96, 3, VH], DT, tag="Zr")
        Zin = plane.tile([96, 3, VH], DT, tag="Zin")
        nc.vector.tensor_mul(Zr[:], Ar_t[o][:], rec[:])
        nc.vector.tensor_mul(Zin[:], Ain_t[o][:], rec[:])
        # I1 (4-group): HrT[v, a'], HiT[v, a']
        HrT = plane.tile([128, 2, 256], DT, tag="HrT", bufs=2)
        HiT = plane.tile([128, 2, 256], DT, tag="HiT", bufs=2)
        for m2 in range(2):
            vc = 128 if m2 == 0 else 17
            vs = slice(m2 * 128, m2 * 128 + vc)
            phr = ps.tile([128, 256], F32, tag="ps", name="ps_hr")
            for k in range(3):
                nc.tensor.matmul(phr[:vc], Zr[:, k, vs], cs['Eur'][:, k, :],
                                 start=(k == 0), stop=False)
            for k in range(3):
                nc.tensor.matmul(phr[:vc], Zin[:, k, vs], cs['Eui'][:, k, :],
                                 start=False, stop=(k == 2))
            copy_ps(HrT[:vc, m2, :], phr[:vc])
            phi = ps.tile([128, 256], F32, tag="ps", name="ps_hi")
            for k in range(3):
                nc.tensor.matmul(phi[:vc], Zr[:, k, vs], cs['Eui'][:, k, :],
                                 start=(k == 0), stop=False)
            for k in range(3):
                nc.tensor.matmul(phi[:vc], Zin[:, k, vs], cs['Eurn'][:, k, :],
                                 start=False, stop=(k == 2))
            copy_ps(HiT[:vc, m2, :], phi[:vc])
        # I2: clear[a, b]
        clearsb = plane.tile([123, 2, 246], DT, tag="clearsb")
        for ma in range(2):
            asl = slice(ma * 123, (ma + 1) * 123)
            pcl = ps.tile([123, 256], F32, tag="ps", name="ps_cl")
            nc.tensor.matmul(pcl[:], HrT[:, 0, asl], cs['wEvr'][:, 0, :],
                             start=True, stop=False)
            nc.tensor.matmul(pcl[:], HrT[:17, 1, asl], cs['wEvr'][:17, 1, :],
                             start=False, stop=False)
            nc.tensor.matmul(pcl[:], HiT[:, 0, asl], cs['wEvin'][:, 0, :],
                             start=False, stop=False)
            nc.tensor.matmul(pcl[:], HiT[:17, 1, asl], cs['wEvin'][:17, 1, :],
                             start=False, stop=True)
            copy_ps(clearsb[:, ma, :], pcl[:, 0:246])
        nc.sync.dma_start(
            _dram_ap(clear_d, o * 246 * 246, [[246, 123], [123 * 246, 2], [1, 246]]),
            clearsb[:])

    emit_den(0)
    for o in range(16):
        if o + 1 < 16:
            emit_den(o + 1)
        emit_ifft(o)
    plane_cm.__exit__(None, None, None)
    astore_cm.__exit__(None, None, None)
    convp_cm.__exit__(None, None, None)
    dump_dram("d_clear", clear_d, 16 * 246 * 246)

    # ---- conv_exp: y[e, p] = sum_o wexpT[o, e] * clear[o, p]
    clear_flat = clear_d.rearrange("o h w -> o (h w)")
    y_flat = y_d.rearrange("e h w -> e (h w)")
    with tc.tile_pool(name="expp", bufs=3) as expp:
        for s in range(nslab):
            j0 = s * SLAB
            jn = min(SLAB, NPIX - j0)
            csl = expp.tile([16, SLAB], DT, tag="clearslab")
            nc.sync.dma_start(csl[:, :jn], clear_flat[:, j0:j0 + jn])
            ysb = expp.tile([64, SLAB], F32, tag="ysb")
            for j in range(0, jn, 512):
                w = min(512, jn - j)
                pt = ps.tile([64, 512], F32, tag="ps", name="ps_exp")
                nc.tensor.matmul(pt[:, :w], wexpT[:], csl[:, j:j + w], start=True, stop=True)
                copy_ps(ysb[:, j:j + w], pt[:, :w])
            nc.gpsimd.dma_start(y_flat[:, j0:j0 + jn], ysb[:, :jn])

    ctx.close()


_NC_CACHE = None
_LAST_RESULT = None
TRACE = False


def _get_nc():
    global _NC_CACHE
    if _NC_CACHE is None:
        _NC_CACHE = build_nc()
    return _NC_CACHE


def kernel(**inputs):
    nc = _get_nc()
    x = np.asarray(inputs['x'], np.float32)
    kerf = np.asarray(inputs['kernel'], np.float32)
    w_red = np.asarray(inputs['w_red'], np.float32)[:, :, 0, 0]     # [16, 64]
    w_g = [np.asarray(inputs[f'w_g{i}'], np.float32) for i in (1, 2, 3)]
    w_g4 = np.asarray(inputs['w_g4'], np.float32)[:, :, 0, 0]       # [16, 16]
    w_exp = np.asarray(inputs['w_exp'], np.float32)[:, :, 0, 0]     # [64, 16]

    shared = {
        'wredT': np.ascontiguousarray(w_red.T, NP_DT),
        'wg4T': np.ascontiguousarray(w_g4.T, NP_DT),
        'wexpT': np.ascontiguousarray(w_exp.T, NP_DT),
    }
    shared['wsh0'] = _wshift_pad(w_g[0])
    shared['wsh1'] = _wshift_pad(w_g[1])
    shared['wsh2'] = _wshift(w_g[2])
    for k, val in CONSTS.items():
        shared[k] = val

    in_maps = []
    for b in range(B):
        m = dict(shared)
        m['x'] = np.ascontiguousarray(x[b], NP_DT)
        m['ker'] = np.ascontiguousarray(kerf[b, 0], NP_DT)
        in_maps.append(m)

    global _LAST_RESULT
    res = run_bass_kernel_spmd(nc, in_maps, core_ids=list(range(B)), trace=TRACE)
    _LAST_RESULT = res
    y = np.stack([res.results[b]['y'] for b in range(B)], axis=0)
    return y.astype(np.float32)


# revision 34
# speedup vs baseline: 1.0392x; 1.0099x over previous
"""Trainium2 Bass kernel for nn_CLS_30562987278491 (Wiener-deconvolution net).

Self-contained: hardcodes shapes B=8, NF=64, C=16, H=W=246, ks=21, FFT N=288.
Sharding: data-parallel over batch B across the 8 NeuronCores (1 image/core).

Decomposition (validated stage-by-stage against the jax reference):
  - conv_red (1x1) as matmul over the channel dim (2048-pixel slabs).
  - 3x3 convs via the R=6 row-shift scheme.  conv1 gathers cls from DRAM;
    h1/h2 stay SBUF-RESIDENT in the gathered rhs layout [(c, rm 0..7), t, x]
    using 128-col zero-padded lhsT (PSUM partitions == rhs partitions, so the
    leaky write is partition-identity) + tiny partition-shift DMAs that fill
    rm 6,7 of slot t from rm 0,1 of slot t+1.  No h1/h2 DRAM round-trips.
  - adaptive pool 3x3 via a [240,3] ones-matmul + free-dim reduce.
  - FFT as DFT matmuls: edge-replication pad folded into Fpad [246,288];
    Hermitian half-spectrum (v < 145).  F2 runs in bf16 (1 cyc/row at 145
    free); the Wiener denominator |Pf|^2 comes from the 5x5 autocorrelation Q
    of kernel_P via f32r matmuls at 290 free (packed QE layout), and Kf2 is
    added on DVE (no identity-matmul).  Plane loop is split in two passes:
    pass A (F1/F2/numerator A = C*conj(Kf)) runs while the pool/kp/Q chain
    resolves; pass B (denominator, Z, IFFT, crop) follows.
  - conv_exp (1x1) as matmul over 2048-pixel slabs.

Perf notes (TimelineSim): 663us baseline -> 451us.  DMA_ENGINES is a single
exclusive resource (~360 GB/s, runs <512B pay 2x); SWDGE costs Pool 994ns
fixed per gpsimd DMA; HWDGE 625ns per sync DMA; f32 matmuls are 4 cyc/row,
f32r 1 cyc/row only at free>=256, bf16 1 cyc/row at any width.  bf16 DMA
(upload or SBUF-SBUF) is BROKEN in this container path - convert on device.
"""
import numpy as np
import ml_dtypes

import concourse.bass as bass
import concourse.bacc as bacc
import concourse.mybir as mybir
import concourse.tile as tile
from concourse.bass_utils import run_bass_kernel_spmd

F32 = mybir.dt.float32
DT = mybir.dt.float32r          # 4-byte, bit-compatible with f32; PE 1 cyc/row at free>=256
BF = mybir.dt.bfloat16
NP_DT = np.float32
NP_BF = ml_dtypes.bfloat16

B, NF, C, H = 8, 64, 16, 246
N = 288
VH = 145                    # N//2 + 1
KS = 21
NPIX = H * H                # 60516
CROP = 21


# ---------------------------------------------------------------- host consts
def _build_consts():
    cs = {}
    u = np.arange(N)
    v = np.arange(VH)
    F = np.exp(-2j * np.pi * np.outer(np.arange(N), u) / N)
    Fpad = np.zeros((H, N), complex)
    Fpad[0] = F[0:22].sum(0)
    Fpad[1:245] = F[22:266]
    Fpad[245] = F[266:288].sum(0)

    FuB = np.concatenate([Fpad.real, Fpad.imag], axis=1)        # [246, 576]
    cs['FuB'] = FuB.reshape(2, 123, 576).transpose(1, 0, 2)     # [123, 2, 576]

    def vchunk(m):                                              # [246,145] -> [123,2,145]
        return m.reshape(2, 123, VH).transpose(1, 0, 2)
    cs['Fvr'] = vchunk(Fpad[:, :VH].real)
    cs['Fvi'] = vchunk(Fpad[:, :VH].imag)
    cs['Fvn'] = vchunk(-Fpad[:, :VH].imag)

    d5 = np.arange(5) - 2
    E5v = np.exp(-2j * np.pi * np.outer(d5, v) / N)             # [5, 145]
    cs['E5v'] = np.concatenate([E5v.real, E5v.imag], axis=1)    # [5, 290]
    th5 = 2 * np.pi * np.outer(d5, u) / N                       # [5, 288]
    cs['E5uc'] = np.cos(th5).reshape(5, 3, 96)
    cs['E5us'] = np.sin(th5).reshape(5, 3, 96)

    d21 = np.arange(21) - 10
    E21u = np.exp(-2j * np.pi * np.outer(d21, u) / N)           # [21, 288]
    cs['E21u'] = np.concatenate([E21u.real, E21u.imag], axis=1)  # [21, 576]
    E21v = np.exp(-2j * np.pi * np.outer(d21, v) / N)           # [21, 145]
    z = np.zeros((21, 290))
    z[:, :VH] = E21v.real
    cs['E21vr'] = z.copy()
    z = np.zeros((21, 290))
    z[:, :VH] = E21v.imag
    cs['E21vi'] = z.copy()
    z = np.zeros((21, 290))
    z[:, :VH] = -E21v.imag
    cs['E21vin'] = z.copy()

    a = CROP + np.arange(256)
    thu = 2 * np.pi * np.outer(u, a) / N                        # [288, 256]
    cs['Eur'] = np.cos(thu).reshape(3, 96, 256).transpose(1, 0, 2)   # [96, 3, 256]
    cs['Eui'] = np.sin(thu).reshape(3, 96, 256).transpose(1, 0, 2)
    cs['Eurn'] = -cs['Eur']

    wv = np.where((v == 0) | (v == N // 2), 1.0, 2.0) / (N * N)
    bb = CROP + np.arange(256)
    thv = 2 * np.pi * np.outer(v, bb) / N                       # [145, 256]
    wEv_r = wv[:, None] * np.cos(thv)
    wEv_i = wv[:, None] * np.sin(thv)
    wEv_r[:, H:] = 0.0
    wEv_i[:, H:] = 0.0

    def vpack(m):                                               # [145,256] -> [128,2,256]
        out = np.zeros((128, 2, 256))
        out[:, 0, :] = m[:128]
        out[:17, 1, :] = m[128:]
        return out
    cs['wEvr'] = vpack(wEv_r)
    cs['wEvin'] = vpack(-wEv_i)

    rows = np.arange(240)
    pt = ((rows[:, None] // 80) == np.arange(3)[None, :]) / 6400.0   # [240, 3]
    cs['poolT'] = pt.reshape(2, 120, 3).transpose(1, 0, 2)      # [120, 2, 3]
    return cs


def _wshift(W):
    """[16,16,3,3] (o,c,dy,dx) -> [128, 3, 96]: [(c,dy'), dx, (o,r)]."""
    ws = np.zeros((128, 3, 96), NP_DT)
    for c in range(16):
        for o in range(16):
            for r in range(6):
                for dy in range(3):
                    ws[c * 8 + r + dy, :, o * 6 + r] += W[o, c, dy, :]
    return ws


def _wshift_pad(W):
    """[16,16,3,3] -> [128, 3, 128]: [(c,dy'), dx, (o,r)] with zero cols r=6,7."""
    ws = np.zeros((128, 3, 128), NP_DT)
    for c in range(16):
        for o in range(16):
            for r in range(6):
                for dy in range(3):
                    ws[c * 8 + r + dy, :, o * 8 + r] += W[o, c, dy, :]
    return ws


_RAW_CONSTS = _build_consts()
# device dtype per const: bf16 for the F2/denominator path, f32r elsewhere
CONST_BF = {'Fvr', 'Fvi', 'Fvn'}
CONST_DT = {'FuB', 'Eur', 'Eui', 'Eurn', 'wEvr', 'wEvin', 'poolT',
            'E21u', 'E21vr', 'E21vi', 'E21vin', 'E5v', 'E5uc', 'E5us'}
CONSTS = {}
for _k, _v in _RAW_CONSTS.items():
    CONSTS[_k] = np.ascontiguousarray(_v, dtype=NP_DT)


# ---------------------------------------------------------------- bass program
def _dram_ap(handle_ap, offset, dims):
    return bass.AP(tensor=handle_ap.tensor, offset=handle_ap.offset + offset, ap=[list(d) for d in dims])


def _sbuf_ap(t, offset, dims):
    return bass.AP(tensor=t.tensor, offset=t.offset + offset, ap=[list(d) for d in dims])


def build_nc():
    nc = bacc.Bacc("TRN2", target_bir_lowering=False, debug=False)

    x_d = nc.dram_tensor("x", [NF, H, H], DT, kind="ExternalInput").ap()
    ker_d = nc.dram_tensor("ker", [21, 21], DT, kind="ExternalInput").ap()
    wredT_d = nc.dram_tensor("wredT", [64, 16], DT, kind="ExternalInput").ap()
    wg4T_d = nc.dram_tensor("wg4T", [16, 16], F32, kind="ExternalInput").ap()
    wexpT_d = nc.dram_tensor("wexpT", [16, 64], DT, kind="ExternalInput").ap()
    wsh_d = [nc.dram_tensor(f"wsh{i}", [128, 3, 128 if i < 2 else 96], DT,
                            kind="ExternalInput").ap() for i in range(3)]
    cd = {}
    for k, val in CONSTS.items():
        cd[k] = nc.dram_tensor(k, list(val.shape), DT, kind="ExternalInput").ap()
    y_d = nc.dram_tensor("y", [NF, H, H], F32, kind="ExternalOutput").ap()
    dbg = {}
    import os as _os
    if _os.environ.get("KDUMP", "0") == "1":
        for nm, shp, dt in [("d_cls", [16, 248, 246], DT), ("d_h1", [16, 248, 244], DT),
                            ("d_h3", [16, 240, 240], DT), ("d_kp", [16, 9], F32),
                            ("d_Kf2", [96, 3, VH], F32), ("d_rec0", [96, 3, VH], F32),
                            ("d_Cr0", [96, 3, VH], F32), ("d_Ci0", [96, 3, VH], F32),
                            ("d_clear", [16, 246, 246], DT),
                            ("d_Q", [16, 25], DT), ("d_QE0", [5, 290], F32),
                            ("d_Qt", [5, 16, 5], F32), ("d_E5v", [5, 290], F32),
                            ("d_E5uc", [5, 3, 96], F32),
                            ("d_P20", [96, 3, VH], F32)]:
            dbg[nm] = nc.dram_tensor(nm, shp, dt, kind="ExternalOutput").ap()

    with tile.TileContext(nc) as tc:
        _emit(nc, tc, x_d, ker_d, wredT_d, wg4T_d, wexpT_d, wsh_d, cd, y_d, dbg)
    nc.compile()
    return nc


def _emit(nc, tc, x_d, ker_d, wredT_d, wg4T_d, wexpT_d, wsh_d, cd, y_d, dbg={}):
    AF = mybir.ActivationFunctionType
    OP = mybir.AluOpType

    def dump_dram(nm, src_d, nelem):
        if nm not in dbg:
            return
        nc.sync.dma_start(
            bass.AP(tensor=dbg[nm].tensor, offset=dbg[nm].offset, ap=[[1, nelem]]),
            bass.AP(tensor=src_d.tensor, offset=src_d.offset, ap=[[1, nelem]]))

    def dump_sbuf(nm, t):
        if nm not in dbg:
            return
        nc.sync.dma_start(dbg[nm][:], t[:])

    import contextlib
    ctx = contextlib.ExitStack()
    consts = ctx.enter_context(tc.tile_pool(name="consts", bufs=1))
    singles = ctx.enter_context(tc.tile_pool(name="singles", bufs=1))
    ps = ctx.enter_context(tc.tile_pool(name="ps", bufs=8, space="PSUM"))
    dram = ctx.enter_context(tc.tile_pool(name="dram", bufs=1, space="DRAM"))

    _cp = [0]

    def copy_ps(dst, src):
        _cp[0] += 1
        if _cp[0] % 2 == 0:
            nc.vector.tensor_copy(dst, src)
        else:
            nc.scalar.activation(dst, src, AF.Copy)

    # ---- conv-critical consts first (sync/SP queue)
    wredT = consts.tile([64, 16], DT)
    nc.sync.dma_start(wredT[:], wredT_d[:])
    wsh = []
    for i in range(3):
        t = consts.tile([128, 3, 128 if i < 2 else 96], DT, name=f"wsh_sb{i}")
        nc.sync.dma_start(t[:], wsh_d[i][:])
        wsh.append(t)
    kersb = consts.tile([21, 21], DT)
    nc.sync.dma_start(kersb[:], ker_d[:])

    # ---- remaining consts; bf16 ones are uploaded f32 and converted on device
    # (bf16 DRAM uploads corrupt partitions >= 3 through this container's
    # PJRT path, so never DMA bf16 from DRAM)
    cs = {}
    with tc.tile_pool(name="bfstage", bufs=1) as bfstage:
        for k, ap_ in cd.items():
            if k in CONST_BF:
                t = bfstage.tile(list(ap_.shape), ap_.dtype, name=f"c_{k}")
                nc.gpsimd.dma_start(t[:], ap_[:])
                tb = consts.tile(list(ap_.shape), BF, name=f"cb_{k}")
                nc.scalar.activation(tb[:], t[:], AF.Copy)
                cs[k] = tb
            else:
                t = consts.tile(list(ap_.shape), ap_.dtype, name=f"c_{k}")
                nc.gpsimd.dma_start(t[:], ap_[:])
                cs[k] = t
    wg4T = consts.tile([16, 16], F32)
    nc.gpsimd.dma_start(wg4T[:], wg4T_d[:])
    wexpT = consts.tile([16, 64], DT)
    nc.gpsimd.dma_start(wexpT[:], wexpT_d[:])


    # ---- DRAM scratch
    cls_d = dram.tile([16, 248, 246], DT)
    h3_d = dram.tile([16, 240, 240], DT)
    clear_d = dram.tile([16, 246, 246], DT)

    # zero the pad rows of cls (rows 246-247) and h1 (rows 246-247)
    zpad32 = singles.tile([16, 2, 246], F32)
    nc.vector.memset(zpad32[:], 0.0)
    zpad = singles.tile([16, 2, 246], DT)
    nc.scalar.activation(zpad[:], zpad32[:], mybir.ActivationFunctionType.Copy)
    nc.sync.dma_start(_dram_ap(cls_d, 246 * 246, [[248 * 246, 16], [246, 2], [1, 246]]),
                      zpad[:])

    # ---- conv_red: cls[o, p] = sum_c wredT[c, o] * x[c, p]
    x_flat = x_d.rearrange("c h w -> c (h w)")
    cls_flat = cls_d.rearrange("o h w -> o (h w)")

    dump_dram("d_cls", cls_d, 16 * 248 * 246)

    # ---- Kf via E21 (once per core); E21v* padded to 290 free for f32r rate
    T21 = singles.tile([21, 576], DT)
    for nch in range(2):
        pt = ps.tile([21, 288], F32, tag="ps", name="ps_t21")
        nc.tensor.matmul(pt[:], kersb[:], cs['E21u'][:, nch * 288:(nch + 1) * 288],
                         start=True, stop=True)
        nc.scalar.activation(T21[:, nch * 288:(nch + 1) * 288], pt[:], AF.Copy)
    Kfr = singles.tile([96, 3, VH], F32)
    Kfi = singles.tile([96, 3, VH], F32)
    for m3 in range(3):
        ptr = ps.tile([96, 290], F32, tag="ps", name="ps_kfr")
        nc.tensor.matmul(ptr[:], T21[:, m3 * 96:(m3 + 1) * 96], cs['E21vr'][:],
                         start=True, stop=False)
        nc.tensor.matmul(ptr[:], T21[:, 288 + m3 * 96:288 + (m3 + 1) * 96], cs['E21vin'][:],
                         start=False, stop=True)
        nc.scalar.activation(Kfr[:, m3, :], ptr[:, :VH], AF.Copy)
        pti = ps.tile([96, 290], F32, tag="ps", name="ps_kfi")
        nc.tensor.matmul(pti[:], T21[:, m3 * 96:(m3 + 1) * 96], cs['E21vi'][:],
                         start=True, stop=False)
        nc.tensor.matmul(pti[:], T21[:, 288 + m3 * 96:288 + (m3 + 1) * 96], cs['E21vr'][:],
                         start=False, stop=True)
        nc.scalar.activation(Kfi[:, m3, :], pti[:, :VH], AF.Copy)
    Kf2 = singles.tile([96, 3, VH], F32)
    sqt = singles.tile([96, 3, VH], F32)
    nc.scalar.activation(Kf2[:], Kfr[:], AF.Square)
    nc.scalar.activation(sqt[:], Kfi[:], AF.Square)
    nc.vector.tensor_add(Kf2[:], Kf2[:], sqt[:])
    dump_sbuf("d_Kf2", Kf2)

    # ---- 3x3 conv chain: h1/h2 SBUF-resident in gathered rhs layout
    # [(c, rm 0..7), t-slot, x]; rows 6t+rm; rm 6,7 filled by shift DMAs from
    # slot t+1 rm 0,1.  conv1/conv2 use 128-col zero-padded lhsT so PSUM
    # partitions match the rhs layout (partition-identity leaky writes).
    convp_cm = tc.tile_pool(name="convp", bufs=2)
    convp = convp_cm.__enter__()
    hp_cm = tc.tile_pool(name="hpool", bufs=1)
    hp = hp_cm.__enter__()
    h1rhs = hp.tile([128, 42, 244], DT, name="h1rhs")
    h2rhs = hp.tile([128, 42, 242], DT, name="h2rhs")
    z128 = convp.tile([128, 2, 244], F32, tag="z128", bufs=1)
    nc.vector.memset(z128[:], 0.0)
    nc.vector.tensor_copy(h1rhs[:, 40:42, :], z128[:])
    nc.vector.tensor_copy(h2rhs[:, 40:42, :], z128[:, :, :242])

    def shift78(hrhs, psz, W_out, s0, s1):
        nsl = s1 - s0 + 1
        if nsl <= 0:
            return
        for r in range(2):
            nc.gpsimd.dma_start(
                _sbuf_ap(hrhs, (6 + r) * psz + s0 * W_out,
                         [[8 * psz, 16], [W_out, nsl], [1, W_out]]),
                _sbuf_ap(hrhs, r * psz + (s0 + 1) * W_out,
                         [[8 * psz, 16], [W_out, nsl], [1, W_out]]))

    def leaky_out(dst_slice, pt, c2, W_out, tag):
        ab = convp.tile([128, 2, 244], F32, tag="convab")
        nc.scalar.activation(ab[:, :c2, :W_out],
                             pt[:, :c2 * W_out].rearrange("m (t j) -> m t j", t=c2),
                             AF.Abs, scale=0.45)
        nc.vector.scalar_tensor_tensor(
            out=dst_slice, in0=pt[:, :c2 * W_out].rearrange("m (t j) -> m t j", t=c2),
            scalar=0.55, in1=ab[:, :c2, :W_out], op0=OP.mult, op1=OP.add)

    SLAB = 2048
    nslab = (NPIX + SLAB - 1) // SLAB
    RSLAB = 2048
    nrslab = (NPIX + RSLAB - 1) // RSLAB
    redp_cm = tc.tile_pool(name="redp", bufs=2)
    redp = redp_cm.__enter__()

    def emit_red(s):
        j0 = s * RSLAB
        jn = min(RSLAB, NPIX - j0)
        xs = redp.tile([64, RSLAB], DT, tag="xslab")
        nc.sync.dma_start(xs[:, :jn], x_flat[:, j0:j0 + jn])
        clssb = redp.tile([16, RSLAB], DT, tag="clssb")
        for j in range(0, jn, 512):
            w = min(512, jn - j)
            pt = ps.tile([16, 512], F32, tag="ps", name="ps_red")
            nc.tensor.matmul(pt[:, :w], wredT[:], xs[:, j:j + w], start=True, stop=True)
            copy_ps(clssb[:, j:j + w], pt[:, :w])
        nc.gpsimd.dma_start(cls_flat[:, j0:j0 + jn], clssb[:, :jn])

    # conv1: cls (DRAM) -> h1rhs
    GRP1 = 8
    W1i, W1o = 246, 244
    psz1 = 42 * W1o

    rhs_of = {}

    def emit_c1_gather(gi):
        t0 = gi * GRP1
        cnt = min(GRP1, 41 - t0)
        rhs = convp.tile([128, GRP1, W1i], DT, tag="convrhs")
        row_sz = GRP1 * W1i
        for dy in range(8):
            (nc.gpsimd if dy < 2 else nc.sync).dma_start(
                _sbuf_ap(rhs, dy * row_sz, [[8 * row_sz, 16], [W1i, cnt], [1, W1i]]),
                _dram_ap(cls_d, (6 * t0 + dy) * W1i,
                         [[248 * W1i, 16], [6 * W1i, cnt], [1, W1i]]))
        rhs_of[gi] = rhs

    def emit_c1(gi):
        t0 = gi * GRP1
        cnt = min(GRP1, 41 - t0)
        if gi not in rhs_of:
            emit_c1_gather(gi)
        rhs = rhs_of.pop(gi)
        for tp in range(0, cnt, 2):
            c2 = min(2, cnt - tp)
            pt = ps.tile([128, 2 * W1o], F32, tag="ps", name="ps_c1")
            for dx in range(3):
                nc.tensor.matmul(pt[:, :c2 * W1o], wsh[0][:, dx, :],
                                 rhs[:, tp:tp + c2, dx:dx + W1o],
                                 start=(dx == 0), stop=(dx == 2))
            leaky_out(h1rhs[:, t0 + tp:t0 + tp + c2, :], pt, c2, W1o, "convab1")
        shift78(h1rhs, psz1, W1o, max(0, t0 - 1),
                t0 + cnt - 2 if t0 + cnt < 41 else 40)

    # conv2: h1rhs -> h2rhs (no gather DMAs)
    W2o = 242
    psz2 = 42 * W2o

    def emit_c2(gi):
        t0 = gi * GRP1
        cnt = min(GRP1, 41 - t0)
        for tp in range(0, cnt, 2):
            c2 = min(2, cnt - tp)
            g = t0 + tp
            pt = ps.tile([128, 2 * W2o], F32, tag="ps", name="ps_c2")
            for dx in range(3):
                nc.tensor.matmul(pt[:, :c2 * W2o], wsh[1][:, dx, :],
                                 h1rhs[:, g:g + c2, dx:dx + W2o],
                                 start=(dx == 0), stop=(dx == 2))
            leaky_out(h2rhs[:, g:g + c2, :], pt, c2, W2o, "convab2")
        shift78(h2rhs, psz2, W2o, max(0, t0 - 1),
                t0 + cnt - 2 if t0 + cnt < 41 else 40)

    # conv3: h2rhs -> h3_d (DRAM) via out8 staging
    W3o = 240

    def emit_c3(gi):
        t0 = gi * 8
        cnt = min(8, 40 - t0)
        out8 = convp.tile([96, 8, W3o], DT, tag="convrhs")
        for tp in range(0, cnt, 2):
            c2 = min(2, cnt - tp)
            pt = ps.tile([96, 2 * W3o], F32, tag="ps", name="ps_c3")
            for dx in range(3):
                nc.tensor.matmul(pt[:, :c2 * W3o], wsh[2][:, dx, :],
                                 h2rhs[:, t0 + tp:t0 + tp + c2, dx:dx + W3o],
                                 start=(dx == 0), stop=(dx == 2))
            nc.scalar.activation(out8[:, tp:tp + c2, :],
                                 pt[:, :c2 * W3o].rearrange("m (t j) -> m t j", t=c2),
                                 AF.Copy)
        out_sz = 8 * W3o
        for r in range(6):
            q = nc.gpsimd if r < 3 else nc.sync
            q.dma_start(
                _dram_ap(h3_d, (6 * t0 + r) * W3o,
                         [[240 * W3o, 16], [6 * W3o, cnt], [1, W3o]]),
                _sbuf_ap(out8, r * out_sz,
                         [[6 * out_sz, 16], [W3o, cnt], [1, W3o]]))

    for s in range(nrslab):
        emit_red(s)
    redp_cm.__exit__(None, None, None)
    NG1 = (41 + GRP1 - 1) // GRP1
    for g in range(NG1):
        emit_c1(g)
    clsT_pre = []
    for o in range(2):
        t = consts.tile([123, 2, 246], DT, name=f"clsT_pre{o}")
        nc.sync.dma_start(t[:], _dram_ap(cls_d, o * 248 * 246,
                                         [[246, 123], [123 * 246, 2], [1, 246]]))
        clsT_pre.append(t)
    d3 = 0
    for g in range(NG1):
        emit_c2(g)
        while d3 < 5 and d3 + 1 < g:
            emit_c3(d3)
            d3 += 1
    while d3 < 5:
        emit_c3(d3)
        d3 += 1
    hp_cm.__exit__(None, None, None)

    dump_dram("d_h3", h3_d, 16 * 240 * 240)


    # ---- per-plane FFT: PASS A (F1, F2, numerator A = C * conj(Kf))
    # Runs before the pool/kp/Q chain so that chain overlaps with PE work.
    astore_cm = tc.tile_pool(name="astore", bufs=1)
    astore = astore_cm.__enter__()
    Ar_t = [astore.tile([96, 3, VH], F32, name=f"Ar{o}") for o in range(16)]
    Ain_t = [astore.tile([96, 3, VH], F32, name=f"Ain{o}") for o in range(16)]
    planeA_cm = tc.tile_pool(name="planeA", bufs=3)
    planeA = planeA_cm.__enter__()
    def emit_passA(o):
        if o < 2:
            clsT = clsT_pre[o]
        else:
            clsT = planeA.tile([123, 2, 246], DT, tag="clsT")
            nc.sync.dma_start(clsT[:],
                              _dram_ap(cls_d, o * 248 * 246,
                                       [[246, 123], [123 * 246, 2], [1, 246]]))
        # F1: R1T[w', u] = sum_i cls[i, w'] Fpad[i, u]   (bf16 out for F2)
        R1T = planeA.tile([123, 2, 576], BF, tag="R1T")
        for m in range(2):
            for nch in range(2):
                pt = ps.tile([123, 288], F32, tag="ps", name="ps_f1")
                for k in range(2):
                    nc.tensor.matmul(pt[:], clsT[:, k, m * 123:(m + 1) * 123],
                                     cs['FuB'][:, k, nch * 288:(nch + 1) * 288],
                                     start=(k == 0), stop=(k == 1))
                nc.scalar.activation(R1T[:, m, nch * 288:(nch + 1) * 288], pt[:], AF.Copy)
        # F2 (bf16, half-spectrum): C[u, v] consumed straight from PSUM by the
        # numerator A = C * conj(Kf) (no Cr/Ci SBUF staging).
        tA = planeA.tile([96, 3, VH], F32, tag="tA", bufs=2)
        tB = planeA.tile([96, 3, VH], F32, tag="tB", bufs=2)
        tC = planeA.tile([96, 3, VH], F32, tag="tC", bufs=2)
        tD = planeA.tile([96, 3, VH], F32, tag="tD", bufs=2)
        Ci = planeA.tile([96, 3, VH], F32, tag="Ci", bufs=2)
        for m3 in range(3):
            pcr = ps.tile([96, VH], F32, tag="ps", name="ps_cr")
            for k in range(2):
                nc.tensor.matmul(pcr[:], R1T[:, k, m3 * 96:(m3 + 1) * 96],
                                 cs['Fvr'][:, k, :], start=(k == 0), stop=False)
            for k in range(2):
                nc.tensor.matmul(pcr[:], R1T[:, k, 288 + m3 * 96:288 + (m3 + 1) * 96],
                                 cs['Fvn'][:, k, :], start=False, stop=(k == 1))
            nc.vector.tensor_mul(tA[:, m3, :], pcr[:], Kfr[:, m3, :])
            nc.vector.tensor_mul(tC[:, m3, :], pcr[:], Kfi[:, m3, :])
            pci = ps.tile([96, VH], F32, tag="ps", name="ps_ci")
            for k in range(2):
                nc.tensor.matmul(pci[:], R1T[:, k, m3 * 96:(m3 + 1) * 96],
                                 cs['Fvi'][:, k, :], start=(k == 0), stop=False)
            for k in range(2):
                nc.tensor.matmul(pci[:], R1T[:, k, 288 + m3 * 96:288 + (m3 + 1) * 96],
                                 cs['Fvr'][:, k, :], start=False, stop=(k == 1))
            nc.scalar.activation(Ci[:, m3, :], pci[:], AF.Copy)
            nc.gpsimd.tensor_mul(tB[:, m3, :], Ci[:, m3, :], Kfi[:, m3, :])
            nc.vector.tensor_mul(tD[:, m3, :], Ci[:, m3, :], Kfr[:, m3, :])
        nc.vector.tensor_add(Ar_t[o][:], tA[:], tB[:])
        nc.gpsimd.tensor_tensor(Ain_t[o][:], tC[:], tD[:], mybir.AluOpType.subtract)

    for o in range(11):
        emit_passA(o)

    # ---- adaptive pool -> kp [16, 9]
    P1sb = singles.tile([3, 16, 240], F32)
    for cc in range(8):
        h3t = convp.tile([120, 2, 2, 240], DT, tag="h3t")
        for rc in range(2):
            nc.sync.dma_start(
                h3t[:, rc, :, :],
                _dram_ap(h3_d, cc * 2 * 240 * 240 + rc * 120 * 240,
                         [[240, 120], [240 * 240, 2], [1, 240]]))
        pt = ps.tile([3, 480], F32, tag="ps", name="ps_pool")
        for rc in range(2):
            nc.tensor.matmul(pt[:], cs['poolT'][:, rc, :],
                             h3t[:, rc, :, :].rearrange("p c w -> p (c w)"),
                             start=(rc == 0), stop=(rc == 1))
        nc.scalar.activation(P1sb[:, cc * 2:(cc + 1) * 2, :],
                             pt[:].rearrange("m (c w) -> m c w", c=2), AF.Copy)
    pooled = singles.tile([3, 16, 3], F32)
    nc.vector.tensor_reduce(pooled[:], P1sb[:].rearrange("p c (bx q) -> p c bx q", q=80),
                            axis=mybir.AxisListType.X, op=OP.add)
    pooled_c = singles.tile([16, 9], F32)
    for by in range(3):
        nc.sync.dma_start(pooled_c[:, by * 3:(by + 1) * 3], pooled[by:by + 1, :, :])

    kp = singles.tile([16, 9], F32)
    pt = ps.tile([16, 9], F32, tag="ps", name="ps_kp")
    nc.tensor.matmul(pt[:], wg4T[:], pooled_c[:], start=True, stop=True)
    ekp = singles.tile([16, 9], F32)
    nc.scalar.activation(ekp[:], pt[:], AF.Exp)
    kmean = singles.tile([16, 1], F32)
    nc.vector.tensor_reduce(kmean[:], ekp[:], axis=mybir.AxisListType.X, op=OP.add)
    kmean9 = singles.tile([16, 1], F32)
    nc.scalar.mul(kmean9[:], kmean[:], 1.0 / 9.0)
    nc.vector.tensor_scalar(out=kp[:], in0=ekp[:], scalar1=kmean9[:], scalar2=None,
                            op0=OP.subtract)
    dump_sbuf("d_kp", kp)

    # ---- Q autocorrelation [16, 25] then Qt [5, 16, 5] (bf16)
    Q = singles.tile([16, 25], DT)
    qtmp = singles.tile([16, 9], F32)
    qtmp2 = singles.tile([16, 9], F32)
    kp3 = kp[:].rearrange("o (r c) -> o r c", r=3)
    for dr in range(-2, 3):
        for dc in range(-2, 3):
            r0, r1 = max(0, dr), min(3, 3 + dr)
            c0, c1 = max(0, dc), min(3, 3 + dc)
            nr, ncol = r1 - r0, c1 - c0
            idx = (dr + 2) * 5 + (dc + 2)
            eng = nc.vector if idx % 2 == 0 else nc.gpsimd
            qt2 = qtmp[:, :nr * ncol] if idx % 2 == 0 else qtmp2[:, :nr * ncol]
            eng.tensor_mul(qt2.rearrange("o (r c) -> o r c", r=nr),
                           kp3[:, r0:r1, c0:c1],
                           kp3[:, r0 - dr:r1 - dr, c0 - dc:c1 - dc])
            with nc.allow_low_precision(reason="f32r bits == f32 bits"):
                nc.vector.tensor_reduce(Q[:, idx:idx + 1], qt2,
                                        axis=mybir.AxisListType.X, op=OP.add)
    dump_sbuf("d_Q", Q)
    Qt = singles.tile([5, 16, 5], DT)
    Qv = Q[:].rearrange("o (dr dc) -> o dr dc", dc=5)
    for dc in range(5):
        nc.sync.dma_start(Qt[dc:dc + 1, :, :], Qv[:, :, dc])

    for o in range(11, 16):
        emit_passA(o)
    planeA_cm.__exit__(None, None, None)
    if "d_Qt" in dbg:
        qtb = singles.tile([5, 16, 5], F32, name="qtb")
        nc.scalar.activation(qtb[:], Qt[:], AF.Copy)
        dump_sbuf("d_Qt", qtb)
        e5f = singles.tile([5, 290], F32, name="e5f")
        nc.scalar.activation(e5f[:], cs['E5v'][:], AF.Copy)
        dump_sbuf("d_E5v", e5f)
        e5u = singles.tile([5, 3, 96], F32, name="e5u")
        nc.scalar.activation(e5u[:], cs['E5uc'][:], AF.Copy)
        dump_sbuf("d_E5uc", e5u)


    # ---- per-plane FFT: PASS B (denominator pipelined one plane ahead of IFFT)
    plane_cm = tc.tile_pool(name="plane", bufs=3)
    plane = plane_cm.__enter__()
    rec_of = {}

    def emit_den(o):
        pqe = ps.tile([5, 290], F32, tag="ps", name="ps_qe")
        nc.tensor.matmul(pqe[:], Qt[:, o, :], cs['E5v'][:], start=True, stop=True)
        QE2 = plane.tile([5, 2, 290], DT, tag="QE", bufs=2)
        nc.scalar.activation(QE2[:, 0, :], pqe[:], AF.Copy)
        nc.scalar.activation(QE2[:, 1, 0:VH], pqe[:, VH:290], AF.Copy)
        if o == 0 and "d_QE0" in dbg:
            qef = plane.tile([5, 290], F32, tag="qef", bufs=1)
            nc.scalar.activation(qef[:], QE2[:, 0, :], AF.Copy)
            dump_sbuf("d_QE0", qef)
        rec = plane.tile([96, 3, VH], F32, tag="rec", bufs=2)
        dsb = plane.tile([96, 3, VH], F32, tag="dsb", bufs=2)
        for m3 in range(3):
            pden = ps.tile([96, 290], F32, tag="ps", name="ps_den")
            nc.tensor.matmul(pden[:], cs['E5uc'][:, m3, :], QE2[:, 0, :],
                             start=True, stop=False)
            nc.tensor.matmul(pden[:], cs['E5us'][:, m3, :], QE2[:, 1, :],
                             start=False, stop=True)
            if o == 0 and "d_P20" in dbg:
                p2f = plane.tile([96, 3, VH], F32, tag="p2f", bufs=1, name="p2f")
                nc.scalar.activation(p2f[:, m3, :], pden[:, :VH], AF.Copy)
                if m3 == 2:
                    dump_sbuf("d_P20", p2f)
            nc.vector.tensor_add(dsb[:, m3, :], pden[:, :VH], Kf2[:, m3, :])
            nc.vector.reciprocal_approx_fast(rec[:, m3, :], dsb[:, m3, :])
        if o == 0:
            dump_sbuf("d_rec0", rec)
        rec_of[o] = rec

    def emit_ifft(o):
        rec = rec_of.pop(o)
        # Z = A * rec
        Zr = plane.tile([96, 3, VH], DT, tag="Zr")
        Zin = plane.tile([96, 3, VH], DT, tag="Zin")
        nc.vector.tensor_mul(Zr[:], Ar_t[o][:], rec[:])
        nc.vector.tensor_mul(Zin[:], Ain_t[o][:], rec[:])
        # I1 (4-group): HrT[v, a'], HiT[v, a']
        HrT = plane.tile([128, 2, 256], DT, tag="HrT", bufs=2)
        HiT = plane.tile([128, 2, 256], DT, tag="HiT", bufs=2)
        for m2 in range(2):
            vc = 128 if m2 == 0 else 17
            vs = slice(m2 * 128, m2 * 128 + vc)
            phr = ps.tile([128, 256], F32, tag="ps", name="ps_hr")
            for k in range(3):
                nc.tensor.matmul(phr[:vc], Zr[:, k, vs], cs['Eur'][:, k, :],
                                 start=(k == 0), stop=False)
            for k in range(3):
                nc.tensor.matmul(phr[:vc], Zin[:, k, vs], cs['Eui'][:, k, :],
                                 start=False, stop=(k == 2))
            copy_ps(HrT[:vc, m2, :], phr[:vc])
            phi = ps.tile([128, 256], F32, tag="ps", name="ps_hi")
            for k in range(3):
                nc.tensor.matmul(phi[:vc], Zr[:, k, vs], cs['Eui'][:, k, :],
                                 start=(k == 0), stop=False)
            for k in range(3):
                nc.tensor.matmul(phi[:vc], Zin[:, k, vs], cs['Eurn'][:, k, :],
                                 start=False, stop=(k == 2))
            copy_ps(HiT[:vc, m2, :], phi[:vc])
        # I2: clear[a, b]
        clearsb = plane.tile([123, 2, 246], DT, tag="clearsb")
        for ma in range(2):
            asl = slice(ma * 123, (ma + 1) * 123)
            pcl = ps.tile([123, 256], F32, tag="ps", name="ps_cl")
            nc.tensor.matmul(pcl[:], HrT[:, 0, asl], cs['wEvr'][:, 0, :],
                             start=True, stop=False)
            nc.tensor.matmul(pcl[:], HrT[:17, 1, asl], cs['wEvr'][:17, 1, :],
                             start=False, stop=False)
            nc.tensor.matmul(pcl[:], HiT[:, 0, asl], cs['wEvin'][:, 0, :],
                             start=False, stop=False)
            nc.tensor.matmul(pcl[:], HiT[:17, 1, asl], cs['wEvin'][:17, 1, :],
                             start=False, stop=True)
            copy_ps(clearsb[:, ma, :], pcl[:, 0:246])
        nc.sync.dma_start(
            _dram_ap(clear_d, o * 246 * 246, [[246, 123], [123 * 246, 2], [1, 246]]),
            clearsb[:])

    emit_den(0)
    for o in range(16):
        if o + 1 < 16:
            emit_den(o + 1)
        emit_ifft(o)
    plane_cm.__exit__(None, None, None)
    astore_cm.__exit__(None, None, None)
    convp_cm.__exit__(None, None, None)
    dump_dram("d_clear", clear_d, 16 * 246 * 246)

    # ---- conv_exp: y[e, p] = sum_o wexpT[o, e] * clear[o, p]
    clear_flat = clear_d.rearrange("o h w -> o (h w)")
    y_flat = y_d.rearrange("e h w -> e (h w)")
    with tc.tile_pool(name="expp", bufs=4) as expp:
        for s in range(nslab):
            j0 = s * SLAB
            jn = min(SLAB, NPIX - j0)
            csl = expp.tile([16, SLAB], DT, tag="clearslab")
            nc.sync.dma_start(csl[:, :jn], clear_flat[:, j0:j0 + jn])
            ysb = expp.tile([64, SLAB], F32, tag="ysb")
            for j in range(0, jn, 512):
                w = min(512, jn - j)
                pt = ps.tile([64, 512], F32, tag="ps", name="ps_exp")
                nc.tensor.matmul(pt[:, :w], wexpT[:], csl[:, j:j + w], start=True, stop=True)
                copy_ps(ysb[:, j:j + w], pt[:, :w])
            nc.gpsimd.dma_start(y_flat[:, j0:j0 + jn], ysb[:, :jn])

    ctx.close()


_NC_CACHE = None
_LAST_RESULT = None
TRACE = False


def _get_nc():
    global _NC_CACHE
    if _NC_CACHE is None:
        _NC_CACHE = build_nc()
    return _NC_CACHE


def kernel(**inputs):
    nc = _get_nc()
    x = np.asarray(inputs['x'], np.float32)
    kerf = np.asarray(inputs['kernel'], np.float32)
    w_red = np.asarray(inputs['w_red'], np.float32)[:, :, 0, 0]     # [16, 64]
    w_g = [np.asarray(inputs[f'w_g{i}'], np.float32) for i in (1, 2, 3)]
    w_g4 = np.asarray(inputs['w_g4'], np.float32)[:, :, 0, 0]       # [16, 16]
    w_exp = np.asarray(inputs['w_exp'], np.float32)[:, :, 0, 0]     # [64, 16]

    shared = {
        'wredT': np.ascontiguousarray(w_red.T, NP_DT),
        'wg4T': np.ascontiguousarray(w_g4.T, NP_DT),
        'wexpT': np.ascontiguousarray(w_exp.T, NP_DT),
    }
    shared['wsh0'] = _wshift_pad(w_g[0])
    shared['wsh1'] = _wshift_pad(w_g[1])
    shared['wsh2'] = _wshift(w_g[2])
    for k, val in CONSTS.items():
        shared[k] = val

    in_maps = []
    for b in range(B):
        m = dict(shared)
        m['x'] = np.ascontiguousarray(x[b], NP_DT)
        m['ker'] = np.ascontiguousarray(kerf[b, 0], NP_DT)
        in_maps.append(m)

    global _LAST_RESULT
    res = run_bass_kernel_spmd(nc, in_maps, core_ids=list(range(B)), trace=TRACE)
    _LAST_RESULT = res
    y = np.stack([res.results[b]['y'] for b in range(B)], axis=0)
    return y.astype(np.float32)
